# revision 52
# baseline (speedup 1.0000x reference)
"""MoE-routed K-cluster autoencoder kernel for 8 Trainium2 NeuronCores.

Strategy
--------
Each row of x is reconstructed by the autoencoder of its kmeans cluster.
Computing all K experts densely for every row (like the reference) does
10x the needed matmul work, so we *route*.

Structured path (default for ~uniform labels): the label histogram is
known at call time, so slot capacities are fitted to it.  The two
largest clusters are split 4 ways into the per-core slot 3 (4 cores
each); the remaining 8 clusters are each *owned* by one core and span
that core's slots 0-2, which share a single weight load.  Per-core slot
capacities (1024, 1024, M-2048, ceil(maxbig/4)) give ~4170 row-slots vs
4608 for the old fixed-capacity config (-10% PE work and x/y bytes), and
owning one cluster per core halves the weight DMA (2 unique clusters per
core instead of 4).  Chunks are ~512 columns (one PSUM bank) to minimize
LDWEIGHTS re-issues and PSUM->SBUF eviction instruction count.

The kernel runs near the per-core DMA throughput ceiling, so all x tiles
are DMA'd up front (they fit in SBUF) and the device program is a
2-deep software pipeline: iteration i runs slot i's decoder-2 stripes
as the PE backbone, with slot i+1's serial e1..d1 chain (one layer per
stripe boundary, so stripe work hides each PSUM-eviction latency) and
slot i+2's encoder-0 units interleaved between stripes.  The final slot
therefore exposes only its own PE-paced d2 + one stripe's DMA.  Decoder
output is drained stripe-major into pair tiles and DMA'd out as soon as
evicted (the last slot flushes single stripes).  Slot 0's first x chunk
lands in two k-split DMAs so e0 can start after ~half the chunk arrives,
and a short pre-warm matmul burst keeps PE activity sustained while the
head DMAs land (HAM clock gate).  PSUM->SBUF bias+ReLU evictions
alternate between ScalarE and VectorE; bf16 operands end-to-end
(~5.6e-3 scale-relative error).

Fallback path (skewed/degenerate labels): the original fixed-capacity
slot config search, unchanged.
"""

import numpy as np

import concourse.tile as tile
from concourse import bacc, mybir
from concourse.bass_utils import run_bass_kernel_spmd

N_CORES = 8
B, D, H1, H2, L, K = 32768, 784, 256, 64, 16, 10
P = 112          # partition tile height for the D axis: 784 = 7 * 112
KT = D // P      # 7 k-tiles along D

# packed weight layout (column offsets in a [128, WSLOT] block).
# e0 is m-major (two 896-col halves).  e2/d0 are stored as 2-chunk
# block-diagonal pair blocks (chunk A on the low partitions, chunk B on
# the high ones); d1 is duplicated on partitions 64:128 so a chunk-B rhs
# living at partition offset 64 can use a partition-aligned lhsT.  The
# single-chunk path reads the low-partition prefix of each block, so the
# same layout serves both.
_E0, _E1, _E2, _D0, _D1, _D2 = 0, 1792, 1920, 1952, 2080, 2336
WSLOT = 3904     # = 1792 + 128 + 32 + 128 + 256 + 1568
BSLOT = 14       # bias columns per block: 2 + 1 + 1 + 1 + 2 + 7

# (slots_per_core, rows_per_slot) fallback configs
_CONFIGS = [(4, 1152), (4, 1280), (8, 640), (16, 320), (32, 160)]

_F32 = mybir.dt.float32
_F32R = mybir.dt.float32r
_BF16 = mybir.dt.bfloat16
_RELU = mybir.ActivationFunctionType.Relu

MODE = "bf16"


def _slot_chunks(C):
    """Split C columns into chunks of <=512 (one PSUM bank) such that
    consecutive chunk PAIRS are equal-sized (the mid chain processes
    chunk pairs as block-diagonal matmuls over a shared tile, which
    requires equal widths within a pair).  C must be even."""
    n = max(1, (C + 511) // 512)
    sizes = []
    rem, nl = C, n
    while nl >= 2:
        a = (rem + nl - 1) // nl
        sizes += [a, a]
        rem -= 2 * a
        nl -= 2
    if nl:
        sizes.append(rem)
    return sizes


# ---------------------------------------------------------------------------
# structured program: caps per slot, slots 0-2 share weight block 0,
# slot 3 uses weight block 1.  x chunk-flattened; y stripe-major.
# ---------------------------------------------------------------------------

def _build_program_structured(caps):
    S = len(caps)
    chunk_lists = [_slot_chunks(c) for c in caps]
    XQ_BUFS = sum(len(cl) for cl in chunk_lists)
    nflat = KT * sum(caps)
    nc = bacc.Bacc("TRN2", target_bir_lowering=False, debug=False)
    xt = nc.dram_tensor("xt", [P, nflat], _BF16, kind="ExternalInput").ap()
    wp = nc.dram_tensor("wp", [128, 2 * WSLOT], _BF16, kind="ExternalInput").ap()
    bp = nc.dram_tensor("bp", [128, 2 * BSLOT], _F32, kind="ExternalInput").ap()
    yt = nc.dram_tensor("yt", [P, nflat], _BF16, kind="ExternalOutput").ap()

    # per-slot x column offsets (in xt/yt, units of columns)
    slot_off = []
    cum = 0
    for c in caps:
        slot_off.append(cum)
        cum += KT * c

    # slot -> weight/bias block: all but the last slot share the owned
    # cluster's weights (block 0); the last holds the big cluster's
    wblk = [0] * (S - 1) + [1]

    with tile.TileContext(nc) as tc:
        with (
            tc.tile_pool(name="wpool", bufs=1) as wpool,
            tc.tile_pool(name="iopool", bufs=1) as iopool,
            tc.tile_pool(name="apool", bufs=1) as apool,
            tc.tile_pool(name="pspool", bufs=1, space="PSUM") as pspool,
        ):
            bsb = wpool.tile([128, 2 * BSLOT], _F32, tag="b", name="bsb", bufs=1)
            wA = wpool.tile([128, WSLOT], _BF16, tag="wA", name="wA", bufs=1)
            wB = wpool.tile([128, WSLOT], _BF16, tag="wB", name="wB", bufs=1)

            xq = {}  # (s, ci) -> tile
            dma_log = []  # (key, bytes) in enqueue (=FIFO service) order

            def fetch_chunk_x(s, ci, eng, ksplit=False):
                """One DMA tile per compute chunk; optional k-split (4+3)
                so e0 can start after ~half the chunk lands."""
                nch = chunk_lists[s][ci]
                off = slot_off[s] + KT * sum(chunk_lists[s][:ci])
                t = iopool.tile([128, KT, nch], _BF16, tag="xq",
                                name="xq", bufs=XQ_BUFS)
                src = xt[:, off:off + KT * nch].rearrange(
                    "p (k n) -> p k n", k=KT)
                if ksplit:
                    eng.dma_start(out=t[0:P, 0:4], in_=src[:, 0:4])
                    dma_log.append((('x', s, ci, 0), P * 4 * nch * 2))
                    eng.dma_start(out=t[0:P, 4:KT], in_=src[:, 4:KT])
                    dma_log.append((('x', s, ci, 1), P * (KT - 4) * nch * 2))
                else:
                    eng.dma_start(out=t[0:P], in_=src)
                    dma_log.append((('x', s, ci, 0), P * KT * nch * 2))
                xq[(s, ci)] = t

            def fetch_slot_x(s, eng, ksplit_first=True):
                if s >= S:
                    return
                for ci in range(len(chunk_lists[s])):
                    fetch_chunk_x(s, ci, eng, ksplit=(ksplit_first and ci == 0))

            # PE pre-warm: throwaway matmuls sustain PE activity from the
            # earliest possible instant so the HAM clock gate (half-rate
            # PE until ~3us of sustained activity) opens before real e0
            # work arrives.
            wu = wpool.tile([128, 512], _BF16, tag="wu", name="wu", bufs=1)
            nc.gpsimd.memset(wu, 0)

            # DMA strategy: ONE ring (sync/q1), everything enqueued in
            # need-order.  Outstanding DMAs on a ring are serviced with a
            # strong bias toward enqueue order, and a second active ring
            # steals ~half the 16 shared SDMA engines for as long as it has
            # work, so the fastest way to feed the critical path is a
            # single FIFO ordered by first-use time.  Weight slabs are
            # split into m-major e0 halves / mid-layer / d2 segments so
            # each lands just before its first consumer; the first chunk
            # of every slot is k-split (4+3) so its e0 can start after
            # ~half the chunk arrives.
            def w_seg(dst, lo, hi, blk=0, key=None):
                nc.sync.dma_start(out=dst[:, lo:hi],
                                  in_=wp[:, blk * WSLOT + lo:blk * WSLOT + hi])
                dma_log.append((('w', blk, key), 128 * (hi - lo) * 2))

            fetch_chunk_x(0, 0, nc.sync, ksplit=True)
            w_seg(wA, _E0, _E0 + 896, key='e0m0')
            w_seg(wA, _E0 + 896, _E1, key='e0m1')
            nc.sync.dma_start(out=bsb, in_=bp)
            dma_log.append((('b',), 128 * 2 * BSLOT * 4))
            for ci in range(1, len(chunk_lists[0])):
                fetch_chunk_x(0, ci, nc.sync)
            w_seg(wA, _E1, _D2, key='mid')     # e1/e2/d0/d1
            fetch_chunk_x(1, 0, nc.sync, ksplit=True)
            w_seg(wA, _D2, WSLOT, key='d2')    # d2
            for ci in range(1, len(chunk_lists[1])):
                fetch_chunk_x(1, ci, nc.sync)
            fetch_chunk_x(2, 0, nc.sync, ksplit=True)
            for ci in range(1, len(chunk_lists[2])):
                fetch_chunk_x(2, ci, nc.sync)
            w_seg(wB, _E0, _E0 + 896, blk=1, key='e0m0')
            w_seg(wB, _E0 + 896, _E1, blk=1, key='e0m1')
            fetch_chunk_x(3, 0, nc.sync, ksplit=True)
            w_seg(wB, _E1, _D2, blk=1, key='mid')
            w_seg(wB, _D2, WSLOT, blk=1, key='d2')
            for ci in range(1, len(chunk_lists[3])):
                fetch_chunk_x(3, ci, nc.sync)
            for s5 in range(4, S):
                fetch_slot_x(s5, nc.sync)

            def wt(s):
                return wA if wblk[s] == 0 else wB

            def bias(s, lo, col):
                bb = wblk[s] * BSLOT
                return bsb[0:lo, bb + col:bb + col + 1]

            def ps_tile(parts, nch):
                return pspool.tile([parts, nch], _F32, tag="ps", name="ps",
                                   bufs=5)

            def e0ps_tile(nch):
                # e0 accumulation chains stay open across interleaved work;
                # a dedicated 2-deep pool keeps an open chain from stalling
                # the flow pool's recycling through the in-order PE queue.
                return pspool.tile([128, nch], _F32, tag="e0ps",
                                   name="e0ps", bufs=2)

            drain_i = [0]

            def drain_relu(out, ps, bias_ap):
                drain_i[0] += 1
                if drain_i[0] % 2:
                    nc.scalar.activation(out, ps, _RELU, bias=bias_ap)
                else:
                    nc.vector.tensor_scalar(out, ps, bias_ap, 0.0,
                                            mybir.AluOpType.add,
                                            mybir.AluOpType.max)

            def drain_bias(out, ps, bias_ap):
                drain_i[0] += 1
                if drain_i[0] % 2:
                    nc.scalar.add(out, ps, bias_ap)
                else:
                    nc.vector.tensor_scalar_add(out, ps, bias_ap)

            h1s = {}   # s -> [m][ci] h1 tiles
            a2s = {}   # s -> [m][ci] a2 tiles
            e0ps = {}  # (s, ci, m) -> open psum accumulation tile

            def e0_part(s, ci, m, k0, k1):
                """e0 k-range [k0,k1) of chunk ci, m-half m; evicts at k=KT.
                Weights are m-major: wA[_E0 + m*896 + k*128 ...]."""
                nch = chunk_lists[s][ci]
                w = wt(s)
                if k0 == 0:
                    e0ps[(s, ci, m)] = e0ps_tile(nch)
                ps = e0ps[(s, ci, m)]
                ent = xq[(s, ci)]
                for k in range(k0, k1):
                    wk = w[0:P, _E0 + m * 896 + k * 128:
                           _E0 + m * 896 + (k + 1) * 128]
                    nc.tensor.matmul(ps, wk, ent[0:P, k, :],
                                     start=(k == 0), stop=(k == KT - 1))
                if k1 == KT:
                    t = apool.tile([128, nch], _BF16, tag="h1", name="h1",
                                   bufs=20)
                    drain_relu(t, ps, bias(s, 128, m))
                    h1s.setdefault(s, [{}, {}])[m][ci] = t
                    del e0ps[(s, ci, m)]

            def e0_unit(s, ci, m):
                e0_part(s, ci, m, 0, KT)

            def mid_groups(s):
                """Chunk pair-groups for the mid chain: [(a, b), ...] plus
                a possible trailing single.  Paired chunks are always
                equal-sized (see _slot_chunks)."""
                NCH = len(chunk_lists[s])
                groups = [(ci, ci + 1) for ci in range(0, NCH - 1, 2)]
                if NCH % 2:
                    groups.append((NCH - 1,))
                return groups

            def mid_steps(s):
                """The serial e1->e2->d0->d1 chain of slot s as 5 steps.

                Chunks are processed in PAIRS: chunk A's activations live
                on the low partitions, chunk B's on the high ones, so
                e2/d0 run as single block-diagonal matmuls over the pair
                (half the PE passes) and e1/e2/d0 evict once per pair
                instead of once per chunk.  d1 splits back per chunk; the
                chunk-B rhs sits at partition offset 64, matched by the
                duplicated d1 weights on partitions 64:128."""
                if s >= S:
                    return
                chunks = chunk_lists[s]
                NCH = len(chunks)
                w = wt(s)
                groups = mid_groups(s)
                st = {}

                def e1():
                    h1 = h1s[s]
                    ps = []
                    for g in groups:
                        nch = chunks[g[0]]
                        p = ps_tile(64 * len(g), nch)
                        for gi, ci in enumerate(g):
                            dst = p[64 * gi:64 * (gi + 1)]
                            for k in range(2):
                                wk = w[0:128, _E1 + 64 * k:_E1 + 64 * k + 64]
                                nc.tensor.matmul(dst, wk, h1[k][ci],
                                                 start=(k == 0),
                                                 stop=(k == 1))
                        ps.append(p)
                    st["h2"] = []
                    for g, p in zip(groups, ps):
                        nch = chunks[g[0]]
                        t = apool.tile([64 * len(g), nch], _BF16, tag="h2",
                                       name="h2", bufs=8)
                        drain_relu(t, p, bias(s, 64 * len(g), 2))
                        st["h2"].append(t)

                def e2():
                    ps = []
                    for g, h2 in zip(groups, st["h2"]):
                        nch = chunks[g[0]]
                        p = ps_tile(16 * len(g), nch)
                        wk = w[0:64 * len(g), _E2:_E2 + 16 * len(g)]
                        nc.tensor.matmul(p, wk, h2, start=True, stop=True)
                        ps.append(p)
                    st["z"] = []
                    for g, p in zip(groups, ps):
                        nch = chunks[g[0]]
                        t = apool.tile([16 * len(g), nch], _BF16, tag="z",
                                       name="z", bufs=8)
                        drain_relu(t, p, bias(s, 16 * len(g), 3))
                        st["z"].append(t)

                def d0():
                    ps = []
                    for g, z in zip(groups, st["z"]):
                        nch = chunks[g[0]]
                        p = ps_tile(64 * len(g), nch)
                        wk = w[0:16 * len(g), _D0:_D0 + 64 * len(g)]
                        nc.tensor.matmul(p, wk, z, start=True, stop=True)
                        ps.append(p)
                    st["a1"] = []
                    for g, p in zip(groups, ps):
                        nch = chunks[g[0]]
                        t = apool.tile([64 * len(g), nch], _BF16, tag="a1",
                                       name="a1", bufs=8)
                        drain_relu(t, p, bias(s, 64 * len(g), 4))
                        st["a1"].append(t)

                def d1(m):
                    a2 = a2s.setdefault(s, [[None] * NCH, [None] * NCH])
                    pss = []
                    for g, a1 in zip(groups, st["a1"]):
                        for gi, ci in enumerate(g):
                            nch = chunks[ci]
                            wk = w[64 * gi:64 * (gi + 1),
                                   _D1 + 128 * m:_D1 + 128 * m + 128]
                            p = ps_tile(128, nch)
                            nc.tensor.matmul(p, wk,
                                             a1[64 * gi:64 * (gi + 1)],
                                             start=True, stop=True)
                            pss.append((ci, nch, p))
                    for ci, nch, p in pss:
                        t = apool.tile([128, nch], _BF16, tag="a2",
                                       name="a2", bufs=20)
                        drain_relu(t, p, bias(s, 128, 5 + m))
                        a2[m][ci] = t

                yield e1
                yield e2
                yield d0
                yield lambda: d1(0)
                yield lambda: d1(1)

            def e0_unit_steps(s):
                """First chunk as two k-split half-units (its DMA is
                k-split, so work can start after ~half the chunk lands),
                remaining chunks as whole units."""
                if s >= S:
                    return
                for m in range(2):
                    yield (lambda m=m: e0_part(s, 0, m, 0, 4))
                for m in range(2):
                    yield (lambda m=m: e0_part(s, 0, m, 4, KT))
                for ci in range(1, len(chunk_lists[s])):
                    for m in range(2):
                        yield (lambda ci=ci, m=m: e0_unit(s, ci, m))

            # dedicated psum bank for warmups: a warm matmul must never wait
            # on the flow pool's eviction backlog (it fills exactly those
            # stalls)
            wups = pspool.tile([128, 512], _F32, tag="wups", name="wups",
                               bufs=1)

            def warm_one():
                nc.tensor.matmul(wups, wu[:, 0:128], wu,
                                 start=True, stop=True)

            def d2_stripe(s, mm, state):
                """One d2 output stripe (112 of 784 features) of slot s."""
                chunks = chunk_lists[s]
                NCH = len(chunks)
                w = wt(s)
                a2 = a2s[s]
                Cs = caps[s]
                pair = 1 if s == S - 1 else 2
                half = mm % pair
                if half == 0:
                    nst = min(pair, KT - mm)
                    state['yqs'] = iopool.tile([112, nst, Cs], _BF16,
                                               tag="yq", name="yqs", bufs=8)
                yqs = state['yqs']
                col_off = 0
                ps = [None] * NCH
                for k in range(2):
                    wk = w[0:128, _D2 + 784 * k + 112 * mm:
                           _D2 + 784 * k + 112 * mm + 112]
                    for ci, nch in enumerate(chunks):
                        if k == 0:
                            ps[ci] = ps_tile(112, nch)
                        nc.tensor.matmul(ps[ci], wk, a2[k][ci],
                                         start=(k == 0), stop=(k == 1))
                for ci, nch in enumerate(chunks):
                    drain_bias(yqs[0:P, half, col_off:col_off + nch],
                               ps[ci], bias(s, 112, 7 + mm))
                    col_off += nch
                if half == pair - 1 or mm == KT - 1:
                    lo = (mm // pair) * pair
                    nc.sync.dma_start(
                        out=yt[:, slot_off[s] + lo * Cs:
                               slot_off[s] + (mm + 1) * Cs]
                        .rearrange("p (t n) -> p t n", n=Cs),
                        in_=yqs[0:P])

            # ---- static list scheduler -----------------------------------
            # The PE queue is strictly in-order, so emission order IS the
            # execution order; anything emitted before its DMA lands blocks
            # every later instruction.  Model each DMA's arrival time (FIFO
            # ring at ~296 GB/s from ~8.7us) and PE progress (half clock
            # until the HAM gate opens ~12us), then greedily emit whichever
            # work is ready: e0 first (it tracks the x stream), mid-chain
            # steps next (latency chains -- emit as soon as eviction
            # latency has passed), d2 stripes as the backbone filler, and
            # pure warmup matmuls when nothing else is ready.
            EVL = 900.0
            HAM_T = 12000.0

            arr = {}
            _cum = 0.0
            for i, (key, nb) in enumerate(dma_log):
                _cum += nb
                arr[key] = max(7300.0 + 650.0 * i + 1500.0,
                               8700.0 + _cum / 296.0)

            def xarr(s, ci, half):
                a = arr.get(('x', s, ci, half))
                if a is None:
                    a = arr[('x', s, ci, 0)]
                return a

            def warr(s, key):
                return arr[('w', wblk[s], key)]

            t_pe = [7800.0]
            EV = [7800.0, 7800.0]   # model clocks of the two drain engines
            ev_i = [0]

            def adv(cost):
                t_pe[0] += cost * (2.0 if t_pe[0] < HAM_T else 1.0)

            def note_drains(drains):
                """Model psum evictions: ~0.0126 ns/elem + fixed overhead,
                alternating scalar/vector.  The 6-deep psum pool lets the
                PE run only a bounded lead ahead of the evictors."""
                for elems in drains:
                    e = ev_i[0] % 2
                    ev_i[0] += 1
                    EV[e] = max(EV[e], t_pe[0] + 150.0) \
                        + elems * 0.0126 + 180.0
                t_pe[0] = max(t_pe[0], max(EV) - 2200.0)

            def fill_stall():
                """If the next flow-pool psum alloc would stall the PE on
                eviction backlog, spend the bubble on warmups (keeps the
                HAM clock gate open through eviction-paced stretches)."""
                lead = max(EV) - 2200.0 - t_pe[0]
                n = 0
                while lead > 350.0 and n < 8:
                    warm_one()
                    lead -= 228.0
                    n += 1

            def mmcost(ncols, nmm):
                return ncols / 2.4 + 15.0 * nmm

            # per-slot work state
            e0_items = []   # s -> list of (gate, cost, emit, drains)
            for s in range(S):
                items = []
                nch0 = chunk_lists[s][0]
                for m in range(2):
                    items.append((lambda s=s, m=m:
                                  max(xarr(s, 0, 0), warr(s, 'e0m%d' % m)),
                                  mmcost(4 * nch0, 4),
                                  lambda s=s, m=m: e0_part(s, 0, m, 0, 4),
                                  []))
                for m in range(2):
                    items.append((lambda s=s, m=m:
                                  max(xarr(s, 0, 1), warr(s, 'e0m%d' % m)),
                                  mmcost(3 * nch0, 3),
                                  lambda s=s, m=m: e0_part(s, 0, m, 4, KT),
                                  [128 * nch0]))
                for ci in range(1, len(chunk_lists[s])):
                    nch = chunk_lists[s][ci]
                    for m in range(2):
                        items.append((lambda s=s, ci=ci, m=m:
                                      max(xarr(s, ci, 0),
                                          warr(s, 'e0m%d' % m)),
                                      mmcost(KT * nch, KT),
                                      lambda s=s, ci=ci, m=m:
                                      e0_unit(s, ci, m),
                                      [128 * nch]))
                e0_items.append(items)
            e0_idx = [0] * S
            h1_ready = [None] * S

            mids = [list(mid_steps(s)) for s in range(S)]
            mid_idx = [0] * S
            mid_ready = [None] * S   # gate time for next step
            mid_costs = []
            mid_drains = []
            for s in range(S):
                C, NCH = caps[s], len(chunk_lists[s])
                chs = chunk_lists[s]
                G = mid_groups(s)
                gcols = sum(chs[g[0]] for g in G)
                mid_costs.append([mmcost(2 * C, 2 * NCH),
                                  mmcost(gcols, len(G)),
                                  mmcost(gcols, len(G)),
                                  mmcost(C, NCH), mmcost(C, NCH)])
                mid_drains.append([[64 * len(g) * chs[g[0]] for g in G],
                                   [16 * len(g) * chs[g[0]] for g in G],
                                   [64 * len(g) * chs[g[0]] for g in G],
                                   [128 * n for n in chs],
                                   [128 * n for n in chs]])

            d2_ready = [None] * S
            d2_mm = [0] * S
            d2_state = [dict() for _ in range(S)]

            # y-drain server model: y DMAs sit behind all x on the FIFO
            # ring, so they only start once the inbound stream finishes;
            # after that they drain at ring rate.  Emitting d2 stripes
            # eagerly whenever this server would idle spreads the y
            # production so the post-compute backlog (pure tail time) is
            # minimal.
            X_DONE = max(arr.values())
            y_drain = [X_DONE]

            def note_stripe(s):
                nb = sum(112 * n for n in chunk_lists[s]) * 2
                start = max(y_drain[0], t_pe[0] + EVL, X_DONE)
                y_drain[0] = start + nb / 296.0

            def d2_candidate():
                for s in range(S):
                    if (d2_ready[s] is not None and d2_mm[s] < KT
                            and d2_ready[s] <= t_pe[0]):
                        return s
                return None

            def emit_d2(s):
                d2_stripe(s, d2_mm[s], d2_state[s])
                C, NCH = caps[s], len(chunk_lists[s])
                adv(mmcost(2 * C, 2 * NCH))
                note_drains([112 * n for n in chunk_lists[s]])
                note_stripe(s)
                d2_mm[s] += 1

            while True:
                fill_stall()
                emitted = False
                # 0) drain-driven d2: if the y-drain server is (about to
                # be) idle, a ready d2 stripe jumps the queue -- y bytes
                # produced while the server idles are free, while bytes
                # produced after the last matmul are pure tail time.
                dc = d2_candidate()
                if dc is not None and y_drain[0] < t_pe[0] + 700.0:
                    emit_d2(dc)
                    continue
                # 1) ready e0 (lowest slot first).  Lookahead bound: slot
                # s's e0 only after mid(s-2) is fully emitted (caps live
                # h1/a2 tiles so pool recycling can't cycle through the
                # in-order PE queue).
                for s in range(S):
                    if s >= 2 and mid_idx[s - 2] < 5:
                        continue
                    if e0_idx[s] < len(e0_items[s]):
                        gate, cost, emit, drains = e0_items[s][e0_idx[s]]
                        if gate() <= t_pe[0]:
                            emit()
                            adv(cost)
                            note_drains(drains)
                            e0_idx[s] += 1
                            if e0_idx[s] == len(e0_items[s]):
                                h1_ready[s] = max(t_pe[0], arr[('b',)]) + EVL
                            emitted = True
                            break
                if emitted:
                    continue
                # 2) ready mid step (slot s only after d2(s-2) is fully
                # emitted -- bounds live a2 tiles to ~2 slots)
                for s in range(S):
                    if s >= 2 and d2_mm[s - 2] < KT:
                        continue
                    if mid_idx[s] < 5 and h1_ready[s] is not None:
                        gate = (max(h1_ready[s], warr(s, 'mid'))
                                if mid_idx[s] == 0 else mid_ready[s])
                        if gate <= t_pe[0]:
                            mids[s][mid_idx[s]]()
                            adv(mid_costs[s][mid_idx[s]])
                            note_drains(mid_drains[s][mid_idx[s]])
                            mid_idx[s] += 1
                            mid_ready[s] = t_pe[0] + EVL
                            if mid_idx[s] == 5:
                                d2_ready[s] = max(t_pe[0] + EVL,
                                                  warr(s, 'd2'))
                            emitted = True
                            break
                if emitted:
                    continue
                # 3) d2 stripe backbone
                dc = d2_candidate()
                if dc is not None:
                    emit_d2(dc)
                    continue
                # 4) nothing ready: finished, short stall, or warmup.
                # Never idle the PE for long -- a >=0.5us activity gap can
                # drop the HAM clock gate back to half rate, so fill waits
                # with warmup matmuls.
                gates = []
                remaining = False
                for s in range(S):
                    if e0_idx[s] < len(e0_items[s]):
                        remaining = True
                        if s < 2 or mid_idx[s - 2] == 5:
                            gates.append(e0_items[s][e0_idx[s]][0]())
                    if mid_idx[s] < 5:
                        remaining = True
                        if (h1_ready[s] is not None
                                and (s < 2 or d2_mm[s - 2] == KT)):
                            gates.append(max(h1_ready[s], warr(s, 'mid'))
                                         if mid_idx[s] == 0
                                         else mid_ready[s])
                    if d2_mm[s] < KT:
                        remaining = True
                        if d2_ready[s] is not None:
                            gates.append(d2_ready[s])
                if not remaining:
                    break
                assert gates, "scheduler wedged: work remains but nothing eligible"
                nxt = min(gates)
                if nxt - t_pe[0] < 120.0:
                    t_pe[0] = nxt
                else:
                    warm_one()
                    adv(mmcost(512, 1))
    nc.compile()
    return nc


def _route_structured(labels):
    """Fit (1024, 1024, M-2048, ceil(maxbig/4)) caps to the histogram.

    Returns (caps, core_clusters[8][2], slot_rows[8][4]) or None if the
    distribution doesn't fit the structured layout."""
    counts = np.bincount(labels, minlength=K)
    if len(counts) != K:
        return None
    order = np.argsort(counts, kind="stable")[::-1]
    b1, b2 = int(order[0]), int(order[1])
    owned = [int(c) for c in order[2:]]
    if len(owned) != 8:
        return None
    M = int(counts[owned].max())
    Mb = int(counts[b1])
    C3 = M - 2048
    C3 += C3 & 1     # even caps: _slot_chunks pairs chunks equal-sized
    C4 = -(-Mb // 4)
    C4 += C4 & 1
    if C3 < 256 or C3 > 2048 or C4 < 256 or C4 > 1024:
        return None
    caps = (1024, 1024, C3, C4)
    if sum(caps) >= 4608:
        return None

    by_cluster = {}
    srt = np.argsort(labels, kind="stable")
    pos = 0
    for c in np.sort(np.unique(labels)):
        cnt = int(counts[c])
        by_cluster[int(c)] = srt[pos:pos + cnt]
        pos += cnt

    core_clusters = []
    slot_rows = []
    for i in range(N_CORES):
        oc = owned[i]
        bc = b1 if i < 4 else b2
        core_clusters.append((oc, bc))
        rows = by_cluster.get(oc, np.empty(0, np.int64))
        brows = by_cluster.get(bc, np.empty(0, np.int64))
        j = i % 4
        q, r = divmod(len(brows), 4)
        starts = [qq * q + min(qq, r) for qq in range(5)]
        part = brows[starts[j]:starts[j + 1]]
        srows = [rows[0:1024], rows[1024:2048], rows[2048:], part]
        for s in range(len(caps)):
            if len(srows[s]) > caps[s]:
                return None
        slot_rows.append(srows)
    return caps, core_clusters, slot_rows


def _pack_cluster(wpk, bpk, params, blk, c):
    """Pack cluster c's weights/biases into block blk of wpk/bpk."""
    wb, bb = blk * WSLOT, blk * BSLOT
    we0, we1, we2 = params["w_e0"][c], params["w_e1"][c], params["w_e2"][c]
    wd0, wd1, wd2 = params["w_d0"][c], params["w_d1"][c], params["w_d2"][c]
    for k in range(KT):
        for m in range(2):
            wpk[0:P, wb + _E0 + m * 896 + k * 128:
                wb + _E0 + m * 896 + (k + 1) * 128] = \
                we0[P * k:P * (k + 1), 128 * m:128 * (m + 1)]
    for k in range(2):
        wpk[0:128, wb + _E1 + 64 * k: wb + _E1 + 64 * (k + 1)] = \
            we1[128 * k:128 * (k + 1), :]
    # e2/d0 as 2-chunk block-diagonal pair blocks; d1 duplicated on the
    # high partitions (chunk-B rhs at partition offset 64).  The single-
    # chunk path reads the low-partition prefix of each block.
    wpk[0:64, wb + _E2: wb + _E2 + 16] = we2
    wpk[64:128, wb + _E2 + 16: wb + _E2 + 32] = we2
    wpk[0:16, wb + _D0: wb + _D0 + 64] = wd0
    wpk[16:32, wb + _D0 + 64: wb + _D0 + 128] = wd0
    wpk[0:64, wb + _D1: wb + _D1 + 256] = wd1
    wpk[64:128, wb + _D1: wb + _D1 + 256] = wd1
    for k in range(2):
        wpk[0:128, wb + _D2 + 784 * k: wb + _D2 + 784 * (k + 1)] = \
            wd2[128 * k:128 * (k + 1), :]
    be0, be1, be2 = params["b_e0"][c], params["b_e1"][c], params["b_e2"][c]
    bd0, bd1, bd2 = params["b_d0"][c], params["b_d1"][c], params["b_d2"][c]
    bpk[0:128, bb + 0] = be0[0:128]
    bpk[0:128, bb + 1] = be0[128:256]
    # mid-layer biases replicated for the pair tiles (chunk B's half of
    # a pair tile holds the same features at a partition offset)
    bpk[0:64, bb + 2] = be1
    bpk[64:128, bb + 2] = be1
    bpk[0:16, bb + 3] = be2
    bpk[16:32, bb + 3] = be2
    bpk[0:64, bb + 4] = bd0
    bpk[64:128, bb + 4] = bd0
    bpk[0:128, bb + 5] = bd1[0:128]
    bpk[0:128, bb + 6] = bd1[128:256]
    for m in range(KT):
        bpk[0:P, bb + 7 + m] = bd2[P * m:P * (m + 1)]


def _dma_chunks(caps, s):
    """DMA-layout chunks: one tile per compute chunk."""
    return _slot_chunks(caps[s])


def _flatten_x_structured(xcore_t, caps):
    """[D, sum(caps)] feature-major slab -> chunk-flattened [P, KT*sum]."""
    flat = np.empty((P, KT * sum(caps)), np.float32)
    pos = col = 0
    for s, C in enumerate(caps):
        for nch in _dma_chunks(caps, s):
            blk = xcore_t[:, col:col + nch]                   # [784, nch]
            blk = blk.reshape(KT, P, nch).transpose(1, 0, 2)  # [P, KT, nch]
            flat[:, pos:pos + KT * nch] = blk.reshape(P, KT * nch)
            pos += KT * nch
            col += nch
    return flat


def _unflatten_y_structured(yflat, caps):
    """stripe-major [P, KT*sum(caps)] -> row-major [sum(caps), D]."""
    out = np.empty((sum(caps), D), np.float32)
    pos = col = 0
    for C in caps:
        blk = yflat[:, pos:pos + KT * C].reshape(P, KT, C)
        out[col:col + C] = blk.transpose(2, 1, 0).reshape(C, D)
        pos += KT * C
        col += C
    return out


# ---------------------------------------------------------------------------
# fallback path: original fixed-capacity config program (unchanged)
# ---------------------------------------------------------------------------

def _mdt_view(ap, mode):
    return ap.bitcast(_F32R) if mode == "f32r" else ap


def _chunks(R, mode="f32r"):
    if mode == "bf16":
        n = max(1, (R + 511) // 512)
        base, extra = divmod(R, n)
        return [base + (1 if i < extra else 0) for i in range(n)]
    out, rem = [], R
    while rem > 0:
        c = min(512, rem)
        if c == 512 and 0 < rem - c < 256:
            c = max(256, min(512, (rem + 1) // 2))
        out.append(c)
        rem -= c
    return out


def _build_program(S, R, mode):
    mdt = _F32R if mode == "f32r" else _BF16
    idt = _F32 if mode == "f32r" else _BF16
    pipelined = mode == "bf16"
    ncols = S * R
    nflat = ncols * KT
    nc = bacc.Bacc("TRN2", target_bir_lowering=False, debug=False)
    xt = nc.dram_tensor("xt", [P, nflat], idt, kind="ExternalInput").ap()
    wp = nc.dram_tensor("wp", [128, S * WSLOT], idt, kind="ExternalInput").ap()
    bp = nc.dram_tensor("bp", [128, S * BSLOT], _F32, kind="ExternalInput").ap()
    yt = nc.dram_tensor("yt", [P, nflat], idt, kind="ExternalOutput").ap()

    chunks = _chunks(R, mode)
    NCH = len(chunks)
    XQ_BUFS = 3 * NCH if pipelined else NCH + 2
    W_BUFS = 4 if pipelined else 2
    H1_BUFS = 10 if pipelined else 6
    SM_BUFS = 6 if pipelined else 3

    with tile.TileContext(nc) as tc:
        with (
            tc.tile_pool(name="wpool", bufs=1) as wpool,
            tc.tile_pool(name="iopool", bufs=1) as iopool,
            tc.tile_pool(name="apool", bufs=1) as apool,
            tc.tile_pool(name="pspool", bufs=1, space="PSUM") as pspool,
        ):
            bsb = wpool.tile([128, S * BSLOT], _F32, tag="b", name="bsb", bufs=1)
            nc.sync.dma_start(out=bsb, in_=bp)

            wu = wpool.tile([128, 512], _BF16, tag="wu", name="wu", bufs=1)
            nc.vector.memset(wu, 0)
            wups = [pspool.tile([128, 512], _F32, tag="ps", name="wups",
                                bufs=8) for _ in range(4)]
            for i in range(16):
                nc.tensor.matmul(wups[i % 4], wu[:, 0:128], wu,
                                 start=True, stop=True)

            def bias(lo, col):
                return bsb[0:lo, col:col + 1]

            def ps_tile(parts, nch):
                return pspool.tile([parts, nch], _F32, tag="ps", name="ps",
                                   bufs=8)

            drain_i = [0]

            def drain_relu(out, ps, bias_ap):
                drain_i[0] += 1
                if drain_i[0] % 2:
                    nc.scalar.activation(out, ps, _RELU, bias=bias_ap)
                else:
                    nc.vector.tensor_scalar(out, ps, bias_ap, 0.0,
                                            mybir.AluOpType.add,
                                            mybir.AluOpType.max)

            def drain_bias(out, ps, bias_ap):
                drain_i[0] += 1
                if drain_i[0] % 2:
                    nc.scalar.add(out, ps, bias_ap)
                else:
                    nc.vector.tensor_scalar_add(out, ps, bias_ap)

            res = {}

            def ensure_slot(s):
                if s in res or s >= S:
                    return
                w = wpool.tile([128, WSLOT], mdt, tag="w", name="w",
                               bufs=W_BUFS)
                nc.sync.dma_start(
                    out=w[:, 0:_E1],
                    in_=_mdt_view(wp[:, s * WSLOT:s * WSLOT + _E1], mode))
                nc.sync.dma_start(
                    out=w[:, _E1:],
                    in_=_mdt_view(wp[:, s * WSLOT + _E1:(s + 1) * WSLOT], mode))
                offs = []
                cum = s * R * KT
                for nch in chunks:
                    offs.append(cum)
                    cum += nch * KT
                xq = []
                for ci, nch in enumerate(chunks):
                    t = iopool.tile([128, KT, nch], mdt, tag="xq", name="xq",
                                    bufs=XQ_BUFS)
                    nc.sync.dma_start(
                        out=t[0:P],
                        in_=_mdt_view(
                            xt[:, offs[ci]:offs[ci] + KT * nch]
                            .rearrange("p (k n) -> p k n", k=KT), mode))
                    xq.append(t)
                res[s] = {"w": w, "xq": xq, "offs": offs, "bb": s * BSLOT,
                          "h1": [[None] * NCH, [None] * NCH],
                          "e0ps": [None, None]}

            def e0_group(s, m, k):
                r = res[s]
                if k == 0:
                    r["e0ps"][m] = [ps_tile(128, nch) for nch in chunks]
                wk = r["w"][0:P, _E0 + m * 896 + k * 128:
                            _E0 + m * 896 + (k + 1) * 128]
                for ci, nch in enumerate(chunks):
                    nc.tensor.matmul(r["e0ps"][m][ci], wk,
                                     r["xq"][ci][0:P, k, :],
                                     start=(k == 0), stop=(k == KT - 1))
                if k == KT - 1:
                    for ci, nch in enumerate(chunks):
                        t = apool.tile([128, nch], mdt, tag="h1", name="h1",
                                       bufs=H1_BUFS)
                        drain_relu(t, r["e0ps"][m][ci], bias(128, r["bb"] + m))
                        r["h1"][m][ci] = t
                    r["e0ps"][m] = None

            E0_ORDER = [(m, k) for m in range(2) for k in range(KT)]

            if pipelined:
                ensure_slot(0)
                ensure_slot(1)
                r0 = res[0]
                for ci, nch in enumerate(chunks):
                    for m in range(2):
                        ps0 = ps_tile(128, nch)
                        for k in range(KT):
                            wk = r0["w"][0:P, _E0 + m * 896 + k * 128:
                                         _E0 + m * 896 + (k + 1) * 128]
                            nc.tensor.matmul(ps0, wk, r0["xq"][ci][0:P, k, :],
                                             start=(k == 0), stop=(k == KT - 1))
                        t = apool.tile([128, nch], mdt, tag="h1", name="h1",
                                       bufs=H1_BUFS)
                        drain_relu(t, ps0, bias(128, r0["bb"] + m))
                        r0["h1"][m][ci] = t

            for s in range(S):
                if pipelined:
                    ensure_slot(s + 2)
                    filler = iter(E0_ORDER) if s + 1 < S else iter([])
                else:
                    ensure_slot(s)
                    for m, k in E0_ORDER:
                        e0_group(s, m, k)
                    filler = iter([])

                def fill(n):
                    for _ in range(n):
                        mk = next(filler, None)
                        if mk is not None:
                            e0_group(s + 1, *mk)

                r = res[s]
                w, bb, offs, h1 = r["w"], r["bb"], r["offs"], r["h1"]

                ps = [None] * NCH
                for k in range(2):
                    wk = w[0:128, _E1 + 64 * k:_E1 + 64 * k + 64]
                    for ci, nch in enumerate(chunks):
                        if k == 0:
                            ps[ci] = ps_tile(64, nch)
                        nc.tensor.matmul(ps[ci], wk, h1[k][ci],
                                         start=(k == 0), stop=(k == 1))
                h2 = []
                for ci, nch in enumerate(chunks):
                    t = apool.tile([64, nch], mdt, tag="h2", name="h2", bufs=SM_BUFS)
                    drain_relu(t, ps[ci], bias(64, bb + 2))
                    h2.append(t)
                fill(2)

                ps = [None] * NCH
                wk = w[0:64, _E2:_E2 + 16]
                for ci, nch in enumerate(chunks):
                    ps[ci] = ps_tile(16, nch)
                    nc.tensor.matmul(ps[ci], wk, h2[ci], start=True, stop=True)
                z = []
                for ci, nch in enumerate(chunks):
                    t = apool.tile([16, nch], mdt, tag="z", name="z", bufs=SM_BUFS)
                    drain_relu(t, ps[ci], bias(16, bb + 3))
                    z.append(t)
                fill(2)

                ps = [None] * NCH
                wk = w[0:16, _D0:_D0 + 64]
                for ci, nch in enumerate(chunks):
                    ps[ci] = ps_tile(64, nch)
                    nc.tensor.matmul(ps[ci], wk, z[ci], start=True, stop=True)
                a1 = []
                for ci, nch in enumerate(chunks):
                    t = apool.tile([64, nch], mdt, tag="a1", name="a1", bufs=SM_BUFS)
                    drain_relu(t, ps[ci], bias(64, bb + 4))
                    a1.append(t)
                fill(2)

                a2 = [[None] * NCH, [None] * NCH]
                for m in range(2):
                    wk = w[0:64, _D1 + 128 * m:_D1 + 128 * m + 128]
                    ps = [None] * NCH
                    for ci, nch in enumerate(chunks):
                        ps[ci] = ps_tile(128, nch)
                        nc.tensor.matmul(ps[ci], wk, a1[ci],
                                         start=True, stop=True)
                    for ci, nch in enumerate(chunks):
                        t = apool.tile([128, nch], mdt, tag="a2", name="a2",
                                       bufs=7)
                        drain_relu(t, ps[ci], bias(128, bb + 5 + m))
                        a2[m][ci] = t
                    fill(2)

                yq = []
                for ci, nch in enumerate(chunks):
                    yq.append(iopool.tile([128, KT, nch], idt, tag="yq",
                                          name="yq", bufs=NCH + (3 if pipelined else 1)))
                for mm in range(KT):
                    ps = [None] * NCH
                    for k in range(2):
                        wk = w[0:128, _D2 + 784 * k + 112 * mm:
                               _D2 + 784 * k + 112 * mm + 112]
                        for ci, nch in enumerate(chunks):
                            if k == 0:
                                ps[ci] = ps_tile(112, nch)
                            nc.tensor.matmul(ps[ci], wk, a2[k][ci],
                                             start=(k == 0), stop=(k == 1))
                    for ci, nch in enumerate(chunks):
                        drain_bias(yq[ci][0:P, mm, :], ps[ci],
                                   bias(112, bb + 7 + mm))
                    if mm < 4:
                        fill(1)
                fill(14)
                for ci, nch in enumerate(chunks):
                    nc.sync.dma_start(
                        out=yt[:, offs[ci]:offs[ci] + KT * nch]
                        .rearrange("p (k n) -> p k n", k=KT),
                        in_=yq[ci][0:P])
                del res[s]
    nc.compile()
    return nc


_programs = {}


def _get_program(key, builder):
    if key not in _programs:
        _programs[key] = builder()
    return _programs[key]


def _pack_weights(params, slot_clusters):
    S = len(slot_clusters)
    wpk = np.zeros((128, S * WSLOT), np.float32)
    bpk = np.zeros((128, S * BSLOT), np.float32)
    for s, c in enumerate(slot_clusters):
        _pack_cluster(wpk, bpk, params, s, c)
    return wpk, bpk


def _route(labels, mode):
    counts = np.bincount(labels, minlength=K)
    configs = _CONFIGS if mode == "bf16" else _CONFIGS[1:]
    for S, R in configs:
        need = int(np.sum((counts + R - 1) // R))
        if need <= N_CORES * S:
            break
    nslots = N_CORES * S
    order = np.argsort(labels, kind="stable")
    slot_cluster = np.zeros(nslots, np.int64)
    slot_rows = [np.empty(0, np.int64)] * nslots
    si = pos = 0
    for c in range(K):
        cnt = int(counts[c])
        rows_c = order[pos:pos + cnt]
        pos += cnt
        for off in range(0, cnt, R):
            slot_cluster[si] = c
            slot_rows[si] = rows_c[off:off + R]
            si += 1
    return S, R, slot_cluster, slot_rows


def _flatten_xcore(xcore_t, R, chunks):
    ncols = xcore_t.shape[1]
    S = ncols // R
    flat = np.empty((P, ncols * KT), np.float32)
    pos = 0
    for s in range(S):
        col = s * R
        for nch in chunks:
            blk = xcore_t[:, col:col + nch]
            blk = blk.reshape(KT, P, nch).transpose(1, 0, 2)
            flat[:, pos:pos + KT * nch] = blk.reshape(P, KT * nch)
            pos += KT * nch
            col += nch
    return flat


def _unflatten_ycore(yflat, R, chunks):
    ncols = yflat.shape[1] // KT
    S = ncols // R
    out = np.empty((ncols, D), np.float32)
    pos = 0
    for s in range(S):
        col = s * R
        for nch in chunks:
            blk = yflat[:, pos:pos + KT * nch].reshape(P, KT, nch)
            out[col:col + nch] = blk.transpose(2, 1, 0).reshape(nch, D)
            pos += KT * nch
            col += nch
    return out


def _run_structured(x, params, strat, trace):
    import ml_dtypes
    caps, core_clusters, slot_rows = strat
    nc = _get_program(("st",) + tuple(caps),
                      lambda: _build_program_structured(caps))
    ncols = sum(caps)
    in_maps = []
    for i in range(N_CORES):
        xcore = np.zeros((ncols, D), np.float32)
        col = 0
        for s in range(len(caps)):
            rows = slot_rows[i][s]
            if len(rows):
                xcore[col:col + len(rows)] = x[rows]
            col += caps[s]
        wpk = np.zeros((128, 2 * WSLOT), np.float32)
        bpk = np.zeros((128, 2 * BSLOT), np.float32)
        _pack_cluster(wpk, bpk, params, 0, core_clusters[i][0])
        _pack_cluster(wpk, bpk, params, 1, core_clusters[i][1])
        xflat = _flatten_x_structured(np.ascontiguousarray(xcore.T), caps)
        in_maps.append({"xt": xflat.astype(ml_dtypes.bfloat16),
                        "wp": wpk.astype(ml_dtypes.bfloat16),
                        "bp": bpk})
    res = run_bass_kernel_spmd(nc, in_maps, core_ids=list(range(N_CORES)),
                               trace=trace)
    out = np.zeros_like(x)
    for i in range(N_CORES):
        yraw = np.asarray(res.results[i]["yt"]).astype(np.float32)
        ytT = _unflatten_y_structured(yraw, caps)
        col = 0
        for s in range(len(caps)):
            rows = slot_rows[i][s]
            if len(rows):
                out[rows] = ytT[col:col + len(rows)]
            col += caps[s]
    return out, res


def _run_generic(x, params, labels, mode, trace):
    S, R, slot_cluster, slot_rows = _route(labels, mode)
    chunks = _chunks(R, mode)
    nc = _get_program((S, R, mode), lambda: _build_program(S, R, mode))
    in_maps = []
    for i in range(N_CORES):
        xcore = np.zeros((S * R, D), np.float32)
        for s in range(S):
            rows = slot_rows[i * S + s]
            if len(rows):
                xcore[s * R: s * R + len(rows)] = x[rows]
        wpk, bpk = _pack_weights(params, slot_cluster[i * S:(i + 1) * S])
        xflat = _flatten_xcore(np.ascontiguousarray(xcore.T), R, chunks)
        if mode == "bf16":
            import ml_dtypes
            xflat = xflat.astype(ml_dtypes.bfloat16)
            wpk = wpk.astype(ml_dtypes.bfloat16)
        in_maps.append({"xt": xflat, "wp": wpk, "bp": bpk})
    res = run_bass_kernel_spmd(nc, in_maps, core_ids=list(range(N_CORES)),
                               trace=trace)
    out = np.zeros_like(x)
    for i in range(N_CORES):
        yraw = np.asarray(res.results[i]["yt"]).astype(np.float32)
        ytT = _unflatten_ycore(yraw, R, chunks)
        for s in range(S):
            rows = slot_rows[i * S + s]
            if len(rows):
                out[rows] = ytT[s * R: s * R + len(rows)]
    return out, res


def kernel_traced(inputs, trace=False, mode=None):
    if mode is None:
        mode = MODE
    x = np.ascontiguousarray(np.asarray(inputs["x"], dtype=np.float32))
    labels = np.asarray(inputs["kmeans_label"]).astype(np.int64).ravel()
    params = {k: np.asarray(v, dtype=np.float32)
              for k, v in inputs.items() if k not in ("x", "kmeans_label")}

    if mode == "bf16":
        strat = _route_structured(labels)
        if strat is not None:
            return _run_structured(x, params, strat, trace)
    return _run_generic(x, params, labels, mode, trace)


def kernel(**inputs):
    out, _ = kernel_traced(inputs, trace=False)
    return out



# revision 67
# speedup vs baseline: 1.2262x; 1.2262x over previous
"""MoE-routed K-cluster autoencoder kernel for 8 Trainium2 NeuronCores.

Strategy
--------
Each row of x is reconstructed by the autoencoder of its kmeans cluster.
Computing all K experts densely for every row (like the reference) does
10x the needed matmul work, so we *route*.

Structured path (default for ~uniform labels): the label histogram is
known at call time, so slot capacities are fitted to it.  The two
largest clusters are split 4 ways into the per-core slot 3 (4 cores
each); the remaining 8 clusters are each *owned* by one core and span
that core's slots 0-2, which share a single weight load.  Per-core slot
capacities (1024, 1024, M-2048, ceil(maxbig/4)) rounded even.

PE-pass reduction: mid-layer chunks are processed in equal-width PAIRS
sharing a 128-partition tile (chunk A low partitions, B high), so
e2 (64->16) and d0 (16->64) run as single block-diagonal matmuls per
pair (matmul cost on TRN2 is out-free-size cycles regardless of K/M
utilization, so halving the pass count halves their cost), and
e1/e2/d0 evict once per pair.  34 -> 33 passes per column and ~30%
fewer mid-chain eviction instructions.

The device program is built by a static list scheduler that models, at
build time: per-DMA arrival (single FIFO ring, need-ordered enqueues,
~296 B/ns with a pessimistic head), PE progress (half clock until the
HAM gate opens ~12us; N/2.4 ns per matmul), the two eviction engines
(ScalarE/VectorE alternating, ~0.0126 ns/elem), the psum-pool lead
bound, and a y-drain server (y DMAs ride behind all x on the ring).
Emission order IS the in-order PE execution order, so the scheduler
only emits work whose inputs have landed: ready mid-chain steps first
(latency chains), then e0 (paced by the x stream; first chunk of every
slot k-split 4+3 so it can start on half a chunk), then d2 stripes as
backbone -- pulled EARLY whenever the y-drain server would idle, which
minimizes the post-compute y backlog (pure tail time).  Model-predicted
stalls are filled with warmup matmuls into a dedicated psum bank: a
>=0.5us PE gap drops the HAM clock gate to half rate, so the PE is
never allowed to go visibly idle.  Lookahead gates (e0(s) after
mid(s-2), mid(s) after d2(s-2)) bound live tiles so pool recycling
cannot deadlock through the in-order PE queue.

bf16 operands end-to-end (~5.6e-3 scale-relative error vs the 2e-2
gate).  Fallback path (skewed/degenerate labels): the original
fixed-capacity slot config search, updated for the shared weight
layout.
"""

import numpy as np

import concourse.tile as tile
from concourse import bacc, mybir
from concourse.bass_utils import run_bass_kernel_spmd

N_CORES = 8
B, D, H1, H2, L, K = 32768, 784, 256, 64, 16, 10
P = 112          # partition tile height for the D axis: 784 = 7 * 112
KT = D // P      # 7 k-tiles along D

# packed weight layout (column offsets in a [128, WSLOT] block).
# e0 is m-major (two 896-col halves).  e2/d0 are stored as 2-chunk
# block-diagonal pair blocks (chunk A on the low partitions, chunk B on
# the high ones); d1 is duplicated on partitions 64:128 so a chunk-B rhs
# living at partition offset 64 can use a partition-aligned lhsT.  The
# single-chunk path reads the low-partition prefix of each block, so the
# same layout serves both.
_E0, _E1, _E2, _D0, _D1, _D2 = 0, 1792, 1920, 1952, 2080, 2336
WSLOT = 3904     # = 1792 + 128 + 32 + 128 + 256 + 1568
BSLOT = 14       # bias columns per block: 2 + 1 + 1 + 1 + 2 + 7

# (slots_per_core, rows_per_slot) fallback configs
_CONFIGS = [(4, 1152), (4, 1280), (8, 640), (16, 320), (32, 160)]

_F32 = mybir.dt.float32
_F32R = mybir.dt.float32r
_BF16 = mybir.dt.bfloat16
_RELU = mybir.ActivationFunctionType.Relu

MODE = "bf16"


def _slot_chunks(C):
    """Split C columns into chunks of <=512 (one PSUM bank) such that
    consecutive chunk PAIRS are equal-sized (the mid chain processes
    chunk pairs as block-diagonal matmuls over a shared tile, which
    requires equal widths within a pair).  C must be even."""
    n = max(1, (C + 511) // 512)
    sizes = []
    rem, nl = C, n
    while nl >= 2:
        a = (rem + nl - 1) // nl
        sizes += [a, a]
        rem -= 2 * a
        nl -= 2
    if nl:
        sizes.append(rem)
    return sizes


# ---------------------------------------------------------------------------
# structured program: caps per slot, slots 0-2 share weight block 0,
# slot 3 uses weight block 1.  x chunk-flattened; y stripe-major.
# ---------------------------------------------------------------------------

def _build_program_structured(caps):
    S = len(caps)
    chunk_lists = [_slot_chunks(c) for c in caps]
    XQ_BUFS = sum(len(cl) for cl in chunk_lists)
    nflat = KT * sum(caps)
    nc = bacc.Bacc("TRN2", target_bir_lowering=False, debug=False)
    xt = nc.dram_tensor("xt", [P, nflat], _BF16, kind="ExternalInput").ap()
    wp = nc.dram_tensor("wp", [128, 2 * WSLOT], _BF16, kind="ExternalInput").ap()
    bp = nc.dram_tensor("bp", [128, 2 * BSLOT], _F32, kind="ExternalInput").ap()
    yt = nc.dram_tensor("yt", [P, nflat], _BF16, kind="ExternalOutput").ap()

    # per-slot x column offsets (in xt/yt, units of columns)
    slot_off = []
    cum = 0
    for c in caps:
        slot_off.append(cum)
        cum += KT * c

    # slot -> weight/bias block: all but the last slot share the owned
    # cluster's weights (block 0); the last holds the big cluster's
    wblk = [0] * (S - 1) + [1]

    with tile.TileContext(nc) as tc:
        with (
            tc.tile_pool(name="wpool", bufs=1) as wpool,
            tc.tile_pool(name="iopool", bufs=1) as iopool,
            tc.tile_pool(name="apool", bufs=1) as apool,
            tc.tile_pool(name="pspool", bufs=1, space="PSUM") as pspool,
        ):
            bsb = wpool.tile([128, 2 * BSLOT], _F32, tag="b", name="bsb", bufs=1)
            wA = wpool.tile([128, WSLOT], _BF16, tag="wA", name="wA", bufs=1)
            wB = wpool.tile([128, WSLOT], _BF16, tag="wB", name="wB", bufs=1)

            xq = {}  # (s, ci) -> tile
            dma_log = []  # (key, bytes) in enqueue (=FIFO service) order

            def fetch_chunk_x(s, ci, eng, ksplit=False):
                """One DMA tile per compute chunk; optional k-split (4+3)
                so e0 can start after ~half the chunk lands."""
                nch = chunk_lists[s][ci]
                off = slot_off[s] + KT * sum(chunk_lists[s][:ci])
                t = iopool.tile([128, KT, nch], _BF16, tag="xq",
                                name="xq", bufs=XQ_BUFS)
                src = xt[:, off:off + KT * nch].rearrange(
                    "p (k n) -> p k n", k=KT)
                if ksplit:
                    eng.dma_start(out=t[0:P, 0:4], in_=src[:, 0:4])
                    dma_log.append((('x', s, ci, 0), P * 4 * nch * 2))
                    eng.dma_start(out=t[0:P, 4:KT], in_=src[:, 4:KT])
                    dma_log.append((('x', s, ci, 1), P * (KT - 4) * nch * 2))
                else:
                    eng.dma_start(out=t[0:P], in_=src)
                    dma_log.append((('x', s, ci, 0), P * KT * nch * 2))
                xq[(s, ci)] = t

            def fetch_slot_x(s, eng, ksplit_first=True):
                if s >= S:
                    return
                for ci in range(len(chunk_lists[s])):
                    fetch_chunk_x(s, ci, eng, ksplit=(ksplit_first and ci == 0))

            # PE pre-warm: throwaway matmuls sustain PE activity from the
            # earliest possible instant so the HAM clock gate (half-rate
            # PE until ~3us of sustained activity) opens before real e0
            # work arrives.
            wu = wpool.tile([128, 512], _BF16, tag="wu", name="wu", bufs=1)
            nc.gpsimd.memset(wu, 0)

            # DMA strategy: ONE ring (sync/q1), everything enqueued in
            # need-order.  Outstanding DMAs on a ring are serviced with a
            # strong bias toward enqueue order, and a second active ring
            # steals ~half the 16 shared SDMA engines for as long as it has
            # work, so the fastest way to feed the critical path is a
            # single FIFO ordered by first-use time.  Weight slabs are
            # split into m-major e0 halves / mid-layer / d2 segments so
            # each lands just before its first consumer; the first chunk
            # of every slot is k-split (4+3) so its e0 can start after
            # ~half the chunk arrives.
            def w_seg(dst, lo, hi, blk=0, key=None):
                nc.sync.dma_start(out=dst[:, lo:hi],
                                  in_=wp[:, blk * WSLOT + lo:blk * WSLOT + hi])
                dma_log.append((('w', blk, key), 128 * (hi - lo) * 2))

            fetch_chunk_x(0, 0, nc.sync, ksplit=True)
            w_seg(wA, _E0, _E0 + 896, key='e0m0')
            w_seg(wA, _E0 + 896, _E1, key='e0m1')
            nc.sync.dma_start(out=bsb, in_=bp)
            dma_log.append((('b',), 128 * 2 * BSLOT * 4))
            for ci in range(1, len(chunk_lists[0])):
                fetch_chunk_x(0, ci, nc.sync)
            w_seg(wA, _E1, _D2, key='mid')     # e1/e2/d0/d1
            fetch_chunk_x(1, 0, nc.sync, ksplit=True)
            w_seg(wA, _D2, WSLOT, key='d2')    # d2
            for ci in range(1, len(chunk_lists[1])):
                fetch_chunk_x(1, ci, nc.sync)
            for s in range(2, S - 1):          # remaining owned slots
                fetch_slot_x(s, nc.sync)
            w_seg(wB, _E0, _E0 + 896, blk=1, key='e0m0')
            w_seg(wB, _E0 + 896, _E1, blk=1, key='e0m1')
            fetch_chunk_x(S - 1, 0, nc.sync, ksplit=True)
            w_seg(wB, _E1, _D2, blk=1, key='mid')
            w_seg(wB, _D2, WSLOT, blk=1, key='d2')
            for ci in range(1, len(chunk_lists[S - 1])):
                fetch_chunk_x(S - 1, ci, nc.sync)

            def wt(s):
                return wA if wblk[s] == 0 else wB

            def bias(s, lo, col):
                bb = wblk[s] * BSLOT
                return bsb[0:lo, bb + col:bb + col + 1]

            def ps_tile(parts, nch):
                return pspool.tile([parts, nch], _F32, tag="ps", name="ps",
                                   bufs=5)

            def e0ps_tile(nch):
                # e0 accumulation chains stay open across interleaved work;
                # a dedicated 2-deep pool keeps an open chain from stalling
                # the flow pool's recycling through the in-order PE queue.
                return pspool.tile([128, nch], _F32, tag="e0ps",
                                   name="e0ps", bufs=2)

            drain_i = [0]

            def drain_relu(out, ps, bias_ap):
                drain_i[0] += 1
                if drain_i[0] % 2:
                    nc.scalar.activation(out, ps, _RELU, bias=bias_ap)
                else:
                    nc.vector.tensor_scalar(out, ps, bias_ap, 0.0,
                                            mybir.AluOpType.add,
                                            mybir.AluOpType.max)

            def drain_bias(out, ps, bias_ap):
                drain_i[0] += 1
                if drain_i[0] % 2:
                    nc.scalar.add(out, ps, bias_ap)
                else:
                    nc.vector.tensor_scalar_add(out, ps, bias_ap)

            h1s = {}   # s -> [m][ci] h1 tiles
            a2s = {}   # s -> [m][ci] a2 tiles
            e0ps = {}  # (s, ci, m) -> open psum accumulation tile

            def e0_part(s, ci, m, k0, k1):
                """e0 k-range [k0,k1) of chunk ci, m-half m; evicts at k=KT.
                Weights are m-major: wA[_E0 + m*896 + k*128 ...]."""
                nch = chunk_lists[s][ci]
                w = wt(s)
                if k0 == 0:
                    e0ps[(s, ci, m)] = e0ps_tile(nch)
                ps = e0ps[(s, ci, m)]
                ent = xq[(s, ci)]
                for k in range(k0, k1):
                    wk = w[0:P, _E0 + m * 896 + k * 128:
                           _E0 + m * 896 + (k + 1) * 128]
                    nc.tensor.matmul(ps, wk, ent[0:P, k, :],
                                     start=(k == 0), stop=(k == KT - 1))
                if k1 == KT:
                    t = apool.tile([128, nch], _BF16, tag="h1", name="h1",
                                   bufs=20)
                    drain_relu(t, ps, bias(s, 128, m))
                    h1s.setdefault(s, [{}, {}])[m][ci] = t
                    del e0ps[(s, ci, m)]

            def e0_unit(s, ci, m):
                e0_part(s, ci, m, 0, KT)

            def mid_groups(s):
                """Chunk pair-groups for the mid chain: [(a, b), ...] plus
                a possible trailing single.  Paired chunks are always
                equal-sized (see _slot_chunks)."""
                NCH = len(chunk_lists[s])
                groups = [(ci, ci + 1) for ci in range(0, NCH - 1, 2)]
                if NCH % 2:
                    groups.append((NCH - 1,))
                return groups

            def mid_steps(s):
                """The serial e1->e2->d0->d1 chain of slot s as 5 steps.

                Chunks are processed in PAIRS: chunk A's activations live
                on the low partitions, chunk B's on the high ones, so
                e2/d0 run as single block-diagonal matmuls over the pair
                (half the PE passes) and e1/e2/d0 evict once per pair
                instead of once per chunk.  d1 splits back per chunk; the
                chunk-B rhs sits at partition offset 64, matched by the
                duplicated d1 weights on partitions 64:128."""
                if s >= S:
                    return
                chunks = chunk_lists[s]
                NCH = len(chunks)
                w = wt(s)
                groups = mid_groups(s)
                st = {}

                def e1():
                    h1 = h1s[s]
                    ps = []
                    for g in groups:
                        nch = chunks[g[0]]
                        p = ps_tile(64 * len(g), nch)
                        for gi, ci in enumerate(g):
                            dst = p[64 * gi:64 * (gi + 1)]
                            for k in range(2):
                                wk = w[0:128, _E1 + 64 * k:_E1 + 64 * k + 64]
                                nc.tensor.matmul(dst, wk, h1[k][ci],
                                                 start=(k == 0),
                                                 stop=(k == 1))
                        ps.append(p)
                    st["h2"] = []
                    for g, p in zip(groups, ps):
                        nch = chunks[g[0]]
                        t = apool.tile([64 * len(g), nch], _BF16, tag="h2",
                                       name="h2", bufs=8)
                        drain_relu(t, p, bias(s, 64 * len(g), 2))
                        st["h2"].append(t)

                def e2():
                    ps = []
                    for g, h2 in zip(groups, st["h2"]):
                        nch = chunks[g[0]]
                        p = ps_tile(16 * len(g), nch)
                        wk = w[0:64 * len(g), _E2:_E2 + 16 * len(g)]
                        nc.tensor.matmul(p, wk, h2, start=True, stop=True)
                        ps.append(p)
                    st["z"] = []
                    for g, p in zip(groups, ps):
                        nch = chunks[g[0]]
                        t = apool.tile([16 * len(g), nch], _BF16, tag="z",
                                       name="z", bufs=8)
                        drain_relu(t, p, bias(s, 16 * len(g), 3))
                        st["z"].append(t)

                def d0():
                    ps = []
                    for g, z in zip(groups, st["z"]):
                        nch = chunks[g[0]]
                        p = ps_tile(64 * len(g), nch)
                        wk = w[0:16 * len(g), _D0:_D0 + 64 * len(g)]
                        nc.tensor.matmul(p, wk, z, start=True, stop=True)
                        ps.append(p)
                    st["a1"] = []
                    for g, p in zip(groups, ps):
                        nch = chunks[g[0]]
                        t = apool.tile([64 * len(g), nch], _BF16, tag="a1",
                                       name="a1", bufs=8)
                        drain_relu(t, p, bias(s, 64 * len(g), 4))
                        st["a1"].append(t)

                def d1(m):
                    a2 = a2s.setdefault(s, [[None] * NCH, [None] * NCH])
                    pss = []
                    for g, a1 in zip(groups, st["a1"]):
                        for gi, ci in enumerate(g):
                            nch = chunks[ci]
                            wk = w[64 * gi:64 * (gi + 1),
                                   _D1 + 128 * m:_D1 + 128 * m + 128]
                            p = ps_tile(128, nch)
                            nc.tensor.matmul(p, wk,
                                             a1[64 * gi:64 * (gi + 1)],
                                             start=True, stop=True)
                            pss.append((ci, nch, p))
                    for ci, nch, p in pss:
                        t = apool.tile([128, nch], _BF16, tag="a2",
                                       name="a2", bufs=20)
                        drain_relu(t, p, bias(s, 128, 5 + m))
                        a2[m][ci] = t

                yield e1
                yield e2
                yield d0
                yield lambda: d1(0)
                yield lambda: d1(1)

            # dedicated psum bank for warmups: a warm matmul must never wait
            # on the flow pool's eviction backlog (it fills exactly those
            # stalls)
            wups = pspool.tile([128, 512], _F32, tag="wups", name="wups",
                               bufs=1)

            def warm_one():
                nc.tensor.matmul(wups, wu[:, 0:128], wu,
                                 start=True, stop=True)

            def d2_stripe(s, mm, state):
                """One d2 output stripe (112 of 784 features) of slot s."""
                chunks = chunk_lists[s]
                NCH = len(chunks)
                w = wt(s)
                a2 = a2s[s]
                Cs = caps[s]
                pair = 1 if s == S - 1 else 2
                half = mm % pair
                if half == 0:
                    nst = min(pair, KT - mm)
                    state['yqs'] = iopool.tile([112, nst, Cs], _BF16,
                                               tag="yq", name="yqs", bufs=8)
                yqs = state['yqs']
                col_off = 0
                ps = [None] * NCH
                for k in range(2):
                    wk = w[0:128, _D2 + 784 * k + 112 * mm:
                           _D2 + 784 * k + 112 * mm + 112]
                    for ci, nch in enumerate(chunks):
                        if k == 0:
                            ps[ci] = ps_tile(112, nch)
                        nc.tensor.matmul(ps[ci], wk, a2[k][ci],
                                         start=(k == 0), stop=(k == 1))
                for ci, nch in enumerate(chunks):
                    drain_bias(yqs[0:P, half, col_off:col_off + nch],
                               ps[ci], bias(s, 112, 7 + mm))
                    col_off += nch
                if half == pair - 1 or mm == KT - 1:
                    lo = (mm // pair) * pair
                    nc.sync.dma_start(
                        out=yt[:, slot_off[s] + lo * Cs:
                               slot_off[s] + (mm + 1) * Cs]
                        .rearrange("p (t n) -> p t n", n=Cs),
                        in_=yqs[0:P])

            # ---- static list scheduler -----------------------------------
            # The PE queue is strictly in-order, so emission order IS the
            # execution order; anything emitted before its DMA lands blocks
            # every later instruction.  Model each DMA's arrival time (FIFO
            # ring at ~296 GB/s from ~8.7us) and PE progress (half clock
            # until the HAM gate opens ~12us), then greedily emit whichever
            # work is ready: e0 first (it tracks the x stream), mid-chain
            # steps next (latency chains -- emit as soon as eviction
            # latency has passed), d2 stripes as the backbone filler, and
            # pure warmup matmuls when nothing else is ready.
            EVL = 800.0
            HAM_T = 12000.0

            arr = {}
            _cum = 0.0
            for i, (key, nb) in enumerate(dma_log):
                _cum += nb
                # ring rate ~296 B/ns steady-state; the first few MB see
                # startup + cross-core contention jitter, so model them
                # pessimistically -- a too-early e0 emission head-of-line
                # blocks the PE and can drop the HAM clock gate to half
                # rate, while surplus warmup fill is nearly free.
                if _cum <= 3.2e6:
                    t_bw = _cum / 262.0
                else:
                    t_bw = 3.2e6 / 262.0 + (_cum - 3.2e6) / 296.0
                arr[key] = max(7300.0 + 650.0 * i + 1500.0, 8700.0 + t_bw)

            def xarr(s, ci, half):
                a = arr.get(('x', s, ci, half))
                if a is None:
                    a = arr[('x', s, ci, 0)]
                return a

            def warr(s, key):
                return arr[('w', wblk[s], key)]

            t_pe = [7800.0]
            EV = [7800.0, 7800.0]   # model clocks of the two drain engines
            ev_i = [0]

            def adv(cost):
                t_pe[0] += cost * (2.0 if t_pe[0] < HAM_T else 1.0)

            def note_drains(drains):
                """Model psum evictions: ~0.0126 ns/elem + fixed overhead,
                alternating scalar/vector.  The 6-deep psum pool lets the
                PE run only a bounded lead ahead of the evictors."""
                for elems in drains:
                    e = ev_i[0] % 2
                    ev_i[0] += 1
                    EV[e] = max(EV[e], t_pe[0] + 150.0) \
                        + elems * 0.0126 + 180.0
                t_pe[0] = max(t_pe[0], max(EV) - 2200.0)

            def fill_stall():
                """If the next flow-pool psum alloc would stall the PE on
                eviction backlog, spend the bubble on warmups (keeps the
                HAM clock gate open through eviction-paced stretches)."""
                lead = max(EV) - 2200.0 - t_pe[0]
                n = 0
                while lead > 350.0 and n < 8:
                    warm_one()
                    lead -= 228.0
                    n += 1

            def mmcost(ncols, nmm):
                return ncols / 2.4 + 15.0 * nmm

            # per-slot work state
            e0_items = []   # s -> list of (gate, cost, emit, drains)
            for s in range(S):
                items = []
                nch0 = chunk_lists[s][0]
                for m in range(2):
                    items.append((lambda s=s, m=m:
                                  max(xarr(s, 0, 0), warr(s, 'e0m%d' % m)),
                                  mmcost(4 * nch0, 4),
                                  lambda s=s, m=m: e0_part(s, 0, m, 0, 4),
                                  []))
                for m in range(2):
                    items.append((lambda s=s, m=m:
                                  max(xarr(s, 0, 1), warr(s, 'e0m%d' % m)),
                                  mmcost(3 * nch0, 3),
                                  lambda s=s, m=m: e0_part(s, 0, m, 4, KT),
                                  [128 * nch0]))
                for ci in range(1, len(chunk_lists[s])):
                    nch = chunk_lists[s][ci]
                    for m in range(2):
                        items.append((lambda s=s, ci=ci, m=m:
                                      max(xarr(s, ci, 0),
                                          warr(s, 'e0m%d' % m)),
                                      mmcost(KT * nch, KT),
                                      lambda s=s, ci=ci, m=m:
                                      e0_unit(s, ci, m),
                                      [128 * nch]))
                e0_items.append(items)
            e0_idx = [0] * S
            h1_ready = [None] * S

            mids = [list(mid_steps(s)) for s in range(S)]
            mid_idx = [0] * S
            mid_ready = [None] * S   # gate time for next step
            mid_costs = []
            mid_drains = []
            for s in range(S):
                C, NCH = caps[s], len(chunk_lists[s])
                chs = chunk_lists[s]
                G = mid_groups(s)
                gcols = sum(chs[g[0]] for g in G)
                mid_costs.append([mmcost(2 * C, 2 * NCH),
                                  mmcost(gcols, len(G)),
                                  mmcost(gcols, len(G)),
                                  mmcost(C, NCH), mmcost(C, NCH)])
                mid_drains.append([[64 * len(g) * chs[g[0]] for g in G],
                                   [16 * len(g) * chs[g[0]] for g in G],
                                   [64 * len(g) * chs[g[0]] for g in G],
                                   [128 * n for n in chs],
                                   [128 * n for n in chs]])

            d2_ready = [None] * S
            d2_mm = [0] * S
            d2_state = [dict() for _ in range(S)]

            # y-drain server model: y DMAs sit behind all x on the FIFO
            # ring, so they only start once the inbound stream finishes;
            # after that they drain at ring rate.  Emitting d2 stripes
            # eagerly whenever this server would idle spreads the y
            # production so the post-compute backlog (pure tail time) is
            # minimal.
            X_DONE = max(arr.values())
            y_drain = [X_DONE]

            def note_stripe(s):
                nb = sum(112 * n for n in chunk_lists[s]) * 2
                start = max(y_drain[0], t_pe[0] + EVL, X_DONE)
                y_drain[0] = start + nb / 296.0

            def d2_candidate():
                for s in range(S):
                    if (d2_ready[s] is not None and d2_mm[s] < KT
                            and d2_ready[s] <= t_pe[0]):
                        return s
                return None

            def emit_d2(s):
                d2_stripe(s, d2_mm[s], d2_state[s])
                C, NCH = caps[s], len(chunk_lists[s])
                adv(mmcost(2 * C, 2 * NCH))
                note_drains([112 * n for n in chunk_lists[s]])
                note_stripe(s)
                d2_mm[s] += 1

            while True:
                fill_stall()
                emitted = False
                # 0) drain-driven d2: if the y-drain server is (about to
                # be) idle, a ready d2 stripe jumps the queue -- y bytes
                # produced while the server idles are free, while bytes
                # produced after the last matmul are pure tail time.
                dc = d2_candidate()
                if dc is not None:
                    # the last slot's e0/mid chain feeds the final d2
                    # stripes; never let eager-d2 delay it
                    sl = S - 1
                    last_chain_ready = (
                        (e0_idx[sl] < len(e0_items[sl])
                         and (sl < 2 or mid_idx[sl - 2] == 5)
                         and e0_items[sl][e0_idx[sl]][0]() <= t_pe[0])
                        or (mid_idx[sl] < 5 and h1_ready[sl] is not None
                            and (sl < 2 or d2_mm[sl - 2] == KT)
                            and (max(h1_ready[sl], warr(sl, 'mid'))
                                 if mid_idx[sl] == 0 else mid_ready[sl])
                            <= t_pe[0]))
                    thresh = 400.0 if last_chain_ready else 1400.0
                    if y_drain[0] < t_pe[0] + thresh:
                        emit_d2(dc)
                        continue
                # 2) ready mid step (slot s only after d2(s-2) is fully
                # emitted -- bounds live a2 tiles to ~2 slots)
                for s in range(S):
                    if s >= 2 and d2_mm[s - 2] < KT:
                        continue
                    if mid_idx[s] < 5 and h1_ready[s] is not None:
                        gate = (max(h1_ready[s], warr(s, 'mid'))
                                if mid_idx[s] == 0 else mid_ready[s])
                        if gate <= t_pe[0]:
                            mids[s][mid_idx[s]]()
                            adv(mid_costs[s][mid_idx[s]])
                            note_drains(mid_drains[s][mid_idx[s]])
                            mid_idx[s] += 1
                            # d1(m1) reads a1, not d1(m0)'s output -- no
                            # eviction latency between the two d1 steps
                            mid_ready[s] = t_pe[0] +                                 (50.0 if mid_idx[s] == 4 else EVL)
                            if mid_idx[s] == 5:
                                # first stripe's k0 matmuls (reading the
                                # d1(m0) half) cover most of the d1(m1)
                                # eviction latency
                                d2_ready[s] = max(t_pe[0] + 250.0,
                                                  warr(s, 'd2'))
                            emitted = True
                            break
                if emitted:
                    continue
                # 1) ready e0 (lowest slot first).  Lookahead bound: slot
                # s's e0 only after mid(s-2) is fully emitted (caps live
                # h1/a2 tiles so pool recycling can't cycle through the
                # in-order PE queue).
                for s in range(S):
                    if s >= 2 and mid_idx[s - 2] < 5:
                        continue
                    if e0_idx[s] < len(e0_items[s]):
                        gate, cost, emit, drains = e0_items[s][e0_idx[s]]
                        if gate() <= t_pe[0]:
                            emit()
                            adv(cost)
                            note_drains(drains)
                            e0_idx[s] += 1
                            if e0_idx[s] == len(e0_items[s]):
                                h1_ready[s] = max(t_pe[0], arr[('b',)]) + EVL
                            emitted = True
                            break
                if emitted:
                    continue
                # 3) d2 stripe backbone
                dc = d2_candidate()
                if dc is not None:
                    emit_d2(dc)
                    continue
                # 4) nothing ready: finished, short stall, or warmup.
                # Never idle the PE for long -- a >=0.5us activity gap can
                # drop the HAM clock gate back to half rate, so fill waits
                # with warmup matmuls.
                gates = []
                remaining = False
                for s in range(S):
                    if e0_idx[s] < len(e0_items[s]):
                        remaining = True
                        if s < 2 or mid_idx[s - 2] == 5:
                            gates.append(e0_items[s][e0_idx[s]][0]())
                    if mid_idx[s] < 5:
                        remaining = True
                        if (h1_ready[s] is not None
                                and (s < 2 or d2_mm[s - 2] == KT)):
                            gates.append(max(h1_ready[s], warr(s, 'mid'))
                                         if mid_idx[s] == 0
                                         else mid_ready[s])
                    if d2_mm[s] < KT:
                        remaining = True
                        if d2_ready[s] is not None:
                            gates.append(d2_ready[s])
                if not remaining:
                    break
                assert gates, "scheduler wedged: work remains but nothing eligible"
                nxt = min(gates)
                if nxt - t_pe[0] < 120.0:
                    t_pe[0] = nxt
                else:
                    warm_one()
                    adv(mmcost(512, 1))
    nc.compile()
    return nc


def _route_structured(labels):
    """Fit (1024, 1024, M-2048, ceil(maxbig/4)) caps to the histogram.

    Returns (caps, core_clusters[8][2], slot_rows[8][4]) or None if the
    distribution doesn't fit the structured layout."""
    counts = np.bincount(labels, minlength=K)
    if len(counts) != K:
        return None
    order = np.argsort(counts, kind="stable")[::-1]
    b1, b2 = int(order[0]), int(order[1])
    owned = [int(c) for c in order[2:]]
    if len(owned) != 8:
        return None
    M = int(counts[owned].max())
    Mb = int(counts[b1])
    C3 = M - 2048
    C3 += C3 & 1     # even caps: _slot_chunks pairs chunks equal-sized
    C4 = -(-Mb // 4)
    C4 += C4 & 1
    if C3 < 256 or C3 > 2048 or C4 < 256 or C4 > 1024:
        return None
    caps = (1024, 1024, C3, C4)
    if sum(caps) >= 4608:
        return None

    by_cluster = {}
    srt = np.argsort(labels, kind="stable")
    pos = 0
    for c in np.sort(np.unique(labels)):
        cnt = int(counts[c])
        by_cluster[int(c)] = srt[pos:pos + cnt]
        pos += cnt

    core_clusters = []
    slot_rows = []
    for i in range(N_CORES):
        oc = owned[i]
        bc = b1 if i < 4 else b2
        core_clusters.append((oc, bc))
        rows = by_cluster.get(oc, np.empty(0, np.int64))
        brows = by_cluster.get(bc, np.empty(0, np.int64))
        j = i % 4
        q, r = divmod(len(brows), 4)
        starts = [qq * q + min(qq, r) for qq in range(5)]
        part = brows[starts[j]:starts[j + 1]]
        srows = [rows[0:1024], rows[1024:2048], rows[2048:], part]
        for s in range(len(caps)):
            if len(srows[s]) > caps[s]:
                return None
        slot_rows.append(srows)
    return caps, core_clusters, slot_rows


def _pack_cluster(wpk, bpk, params, blk, c):
    """Pack cluster c's weights/biases into block blk of wpk/bpk."""
    wb, bb = blk * WSLOT, blk * BSLOT
    we0, we1, we2 = params["w_e0"][c], params["w_e1"][c], params["w_e2"][c]
    wd0, wd1, wd2 = params["w_d0"][c], params["w_d1"][c], params["w_d2"][c]
    for k in range(KT):
        for m in range(2):
            wpk[0:P, wb + _E0 + m * 896 + k * 128:
                wb + _E0 + m * 896 + (k + 1) * 128] = \
                we0[P * k:P * (k + 1), 128 * m:128 * (m + 1)]
    for k in range(2):
        wpk[0:128, wb + _E1 + 64 * k: wb + _E1 + 64 * (k + 1)] = \
            we1[128 * k:128 * (k + 1), :]
    # e2/d0 as 2-chunk block-diagonal pair blocks; d1 duplicated on the
    # high partitions (chunk-B rhs at partition offset 64).  The single-
    # chunk path reads the low-partition prefix of each block.
    wpk[0:64, wb + _E2: wb + _E2 + 16] = we2
    wpk[64:128, wb + _E2 + 16: wb + _E2 + 32] = we2
    wpk[0:16, wb + _D0: wb + _D0 + 64] = wd0
    wpk[16:32, wb + _D0 + 64: wb + _D0 + 128] = wd0
    wpk[0:64, wb + _D1: wb + _D1 + 256] = wd1
    wpk[64:128, wb + _D1: wb + _D1 + 256] = wd1
    for k in range(2):
        wpk[0:128, wb + _D2 + 784 * k: wb + _D2 + 784 * (k + 1)] = \
            wd2[128 * k:128 * (k + 1), :]
    be0, be1, be2 = params["b_e0"][c], params["b_e1"][c], params["b_e2"][c]
    bd0, bd1, bd2 = params["b_d0"][c], params["b_d1"][c], params["b_d2"][c]
    bpk[0:128, bb + 0] = be0[0:128]
    bpk[0:128, bb + 1] = be0[128:256]
    # mid-layer biases replicated for the pair tiles (chunk B's half of
    # a pair tile holds the same features at a partition offset)
    bpk[0:64, bb + 2] = be1
    bpk[64:128, bb + 2] = be1
    bpk[0:16, bb + 3] = be2
    bpk[16:32, bb + 3] = be2
    bpk[0:64, bb + 4] = bd0
    bpk[64:128, bb + 4] = bd0
    bpk[0:128, bb + 5] = bd1[0:128]
    bpk[0:128, bb + 6] = bd1[128:256]
    for m in range(KT):
        bpk[0:P, bb + 7 + m] = bd2[P * m:P * (m + 1)]


def _dma_chunks(caps, s):
    """DMA-layout chunks: one tile per compute chunk."""
    return _slot_chunks(caps[s])


def _flatten_x_structured(xcore_t, caps):
    """[D, sum(caps)] feature-major slab -> chunk-flattened [P, KT*sum]."""
    flat = np.empty((P, KT * sum(caps)), np.float32)
    pos = col = 0
    for s, C in enumerate(caps):
        for nch in _dma_chunks(caps, s):
            blk = xcore_t[:, col:col + nch]                   # [784, nch]
            blk = blk.reshape(KT, P, nch).transpose(1, 0, 2)  # [P, KT, nch]
            flat[:, pos:pos + KT * nch] = blk.reshape(P, KT * nch)
            pos += KT * nch
            col += nch
    return flat


def _unflatten_y_structured(yflat, caps):
    """stripe-major [P, KT*sum(caps)] -> row-major [sum(caps), D]."""
    out = np.empty((sum(caps), D), np.float32)
    pos = col = 0
    for C in caps:
        blk = yflat[:, pos:pos + KT * C].reshape(P, KT, C)
        out[col:col + C] = blk.transpose(2, 1, 0).reshape(C, D)
        pos += KT * C
        col += C
    return out


# ---------------------------------------------------------------------------
# fallback path: original fixed-capacity config program (unchanged)
# ---------------------------------------------------------------------------

def _mdt_view(ap, mode):
    return ap.bitcast(_F32R) if mode == "f32r" else ap


def _chunks(R, mode="f32r"):
    if mode == "bf16":
        n = max(1, (R + 511) // 512)
        base, extra = divmod(R, n)
        return [base + (1 if i < extra else 0) for i in range(n)]
    out, rem = [], R
    while rem > 0:
        c = min(512, rem)
        if c == 512 and 0 < rem - c < 256:
            c = max(256, min(512, (rem + 1) // 2))
        out.append(c)
        rem -= c
    return out


def _build_program(S, R, mode):
    mdt = _F32R if mode == "f32r" else _BF16
    idt = _F32 if mode == "f32r" else _BF16
    pipelined = mode == "bf16"
    ncols = S * R
    nflat = ncols * KT
    nc = bacc.Bacc("TRN2", target_bir_lowering=False, debug=False)
    xt = nc.dram_tensor("xt", [P, nflat], idt, kind="ExternalInput").ap()
    wp = nc.dram_tensor("wp", [128, S * WSLOT], idt, kind="ExternalInput").ap()
    bp = nc.dram_tensor("bp", [128, S * BSLOT], _F32, kind="ExternalInput").ap()
    yt = nc.dram_tensor("yt", [P, nflat], idt, kind="ExternalOutput").ap()

    chunks = _chunks(R, mode)
    NCH = len(chunks)
    XQ_BUFS = 3 * NCH if pipelined else NCH + 2
    W_BUFS = 4 if pipelined else 2
    H1_BUFS = 10 if pipelined else 6
    SM_BUFS = 6 if pipelined else 3

    with tile.TileContext(nc) as tc:
        with (
            tc.tile_pool(name="wpool", bufs=1) as wpool,
            tc.tile_pool(name="iopool", bufs=1) as iopool,
            tc.tile_pool(name="apool", bufs=1) as apool,
            tc.tile_pool(name="pspool", bufs=1, space="PSUM") as pspool,
        ):
            bsb = wpool.tile([128, S * BSLOT], _F32, tag="b", name="bsb", bufs=1)
            nc.sync.dma_start(out=bsb, in_=bp)

            wu = wpool.tile([128, 512], _BF16, tag="wu", name="wu", bufs=1)
            nc.vector.memset(wu, 0)
            wups = [pspool.tile([128, 512], _F32, tag="ps", name="wups",
                                bufs=8) for _ in range(4)]
            for i in range(16):
                nc.tensor.matmul(wups[i % 4], wu[:, 0:128], wu,
                                 start=True, stop=True)

            def bias(lo, col):
                return bsb[0:lo, col:col + 1]

            def ps_tile(parts, nch):
                return pspool.tile([parts, nch], _F32, tag="ps", name="ps",
                                   bufs=8)

            drain_i = [0]

            def drain_relu(out, ps, bias_ap):
                drain_i[0] += 1
                if drain_i[0] % 2:
                    nc.scalar.activation(out, ps, _RELU, bias=bias_ap)
                else:
                    nc.vector.tensor_scalar(out, ps, bias_ap, 0.0,
                                            mybir.AluOpType.add,
                                            mybir.AluOpType.max)

            def drain_bias(out, ps, bias_ap):
                drain_i[0] += 1
                if drain_i[0] % 2:
                    nc.scalar.add(out, ps, bias_ap)
                else:
                    nc.vector.tensor_scalar_add(out, ps, bias_ap)

            res = {}

            def ensure_slot(s):
                if s in res or s >= S:
                    return
                w = wpool.tile([128, WSLOT], mdt, tag="w", name="w",
                               bufs=W_BUFS)
                nc.sync.dma_start(
                    out=w[:, 0:_E1],
                    in_=_mdt_view(wp[:, s * WSLOT:s * WSLOT + _E1], mode))
                nc.sync.dma_start(
                    out=w[:, _E1:],
                    in_=_mdt_view(wp[:, s * WSLOT + _E1:(s + 1) * WSLOT], mode))
                offs = []
                cum = s * R * KT
                for nch in chunks:
                    offs.append(cum)
                    cum += nch * KT
                xq = []
                for ci, nch in enumerate(chunks):
                    t = iopool.tile([128, KT, nch], mdt, tag="xq", name="xq",
                                    bufs=XQ_BUFS)
                    nc.sync.dma_start(
                        out=t[0:P],
                        in_=_mdt_view(
                            xt[:, offs[ci]:offs[ci] + KT * nch]
                            .rearrange("p (k n) -> p k n", k=KT), mode))
                    xq.append(t)
                res[s] = {"w": w, "xq": xq, "offs": offs, "bb": s * BSLOT,
                          "h1": [[None] * NCH, [None] * NCH],
                          "e0ps": [None, None]}

            def e0_group(s, m, k):
                r = res[s]
                if k == 0:
                    r["e0ps"][m] = [ps_tile(128, nch) for nch in chunks]
                wk = r["w"][0:P, _E0 + m * 896 + k * 128:
                            _E0 + m * 896 + (k + 1) * 128]
                for ci, nch in enumerate(chunks):
                    nc.tensor.matmul(r["e0ps"][m][ci], wk,
                                     r["xq"][ci][0:P, k, :],
                                     start=(k == 0), stop=(k == KT - 1))
                if k == KT - 1:
                    for ci, nch in enumerate(chunks):
                        t = apool.tile([128, nch], mdt, tag="h1", name="h1",
                                       bufs=H1_BUFS)
                        drain_relu(t, r["e0ps"][m][ci], bias(128, r["bb"] + m))
                        r["h1"][m][ci] = t
                    r["e0ps"][m] = None

            E0_ORDER = [(m, k) for m in range(2) for k in range(KT)]

            if pipelined:
                ensure_slot(0)
                ensure_slot(1)
                r0 = res[0]
                for ci, nch in enumerate(chunks):
                    for m in range(2):
                        ps0 = ps_tile(128, nch)
                        for k in range(KT):
                            wk = r0["w"][0:P, _E0 + m * 896 + k * 128:
                                         _E0 + m * 896 + (k + 1) * 128]
                            nc.tensor.matmul(ps0, wk, r0["xq"][ci][0:P, k, :],
                                             start=(k == 0), stop=(k == KT - 1))
                        t = apool.tile([128, nch], mdt, tag="h1", name="h1",
                                       bufs=H1_BUFS)
                        drain_relu(t, ps0, bias(128, r0["bb"] + m))
                        r0["h1"][m][ci] = t

            for s in range(S):
                if pipelined:
                    ensure_slot(s + 2)
                    filler = iter(E0_ORDER) if s + 1 < S else iter([])
                else:
                    ensure_slot(s)
                    for m, k in E0_ORDER:
                        e0_group(s, m, k)
                    filler = iter([])

                def fill(n):
                    for _ in range(n):
                        mk = next(filler, None)
                        if mk is not None:
                            e0_group(s + 1, *mk)

                r = res[s]
                w, bb, offs, h1 = r["w"], r["bb"], r["offs"], r["h1"]

                ps = [None] * NCH
                for k in range(2):
                    wk = w[0:128, _E1 + 64 * k:_E1 + 64 * k + 64]
                    for ci, nch in enumerate(chunks):
                        if k == 0:
                            ps[ci] = ps_tile(64, nch)
                        nc.tensor.matmul(ps[ci], wk, h1[k][ci],
                                         start=(k == 0), stop=(k == 1))
                h2 = []
                for ci, nch in enumerate(chunks):
                    t = apool.tile([64, nch], mdt, tag="h2", name="h2", bufs=SM_BUFS)
                    drain_relu(t, ps[ci], bias(64, bb + 2))
                    h2.append(t)
                fill(2)

                ps = [None] * NCH
                wk = w[0:64, _E2:_E2 + 16]
                for ci, nch in enumerate(chunks):
                    ps[ci] = ps_tile(16, nch)
                    nc.tensor.matmul(ps[ci], wk, h2[ci], start=True, stop=True)
                z = []
                for ci, nch in enumerate(chunks):
                    t = apool.tile([16, nch], mdt, tag="z", name="z", bufs=SM_BUFS)
                    drain_relu(t, ps[ci], bias(16, bb + 3))
                    z.append(t)
                fill(2)

                ps = [None] * NCH
                wk = w[0:16, _D0:_D0 + 64]
                for ci, nch in enumerate(chunks):
                    ps[ci] = ps_tile(64, nch)
                    nc.tensor.matmul(ps[ci], wk, z[ci], start=True, stop=True)
                a1 = []
                for ci, nch in enumerate(chunks):
                    t = apool.tile([64, nch], mdt, tag="a1", name="a1", bufs=SM_BUFS)
                    drain_relu(t, ps[ci], bias(64, bb + 4))
                    a1.append(t)
                fill(2)

                a2 = [[None] * NCH, [None] * NCH]
                for m in range(2):
                    wk = w[0:64, _D1 + 128 * m:_D1 + 128 * m + 128]
                    ps = [None] * NCH
                    for ci, nch in enumerate(chunks):
                        ps[ci] = ps_tile(128, nch)
                        nc.tensor.matmul(ps[ci], wk, a1[ci],
                                         start=True, stop=True)
                    for ci, nch in enumerate(chunks):
                        t = apool.tile([128, nch], mdt, tag="a2", name="a2",
                                       bufs=7)
                        drain_relu(t, ps[ci], bias(128, bb + 5 + m))
                        a2[m][ci] = t
                    fill(2)

                yq = []
                for ci, nch in enumerate(chunks):
                    yq.append(iopool.tile([128, KT, nch], idt, tag="yq",
                                          name="yq", bufs=NCH + (3 if pipelined else 1)))
                for mm in range(KT):
                    ps = [None] * NCH
                    for k in range(2):
                        wk = w[0:128, _D2 + 784 * k + 112 * mm:
                               _D2 + 784 * k + 112 * mm + 112]
                        for ci, nch in enumerate(chunks):
                            if k == 0:
                                ps[ci] = ps_tile(112, nch)
                            nc.tensor.matmul(ps[ci], wk, a2[k][ci],
                                             start=(k == 0), stop=(k == 1))
                    for ci, nch in enumerate(chunks):
                        drain_bias(yq[ci][0:P, mm, :], ps[ci],
                                   bias(112, bb + 7 + mm))
                    if mm < 4:
                        fill(1)
                fill(14)
                for ci, nch in enumerate(chunks):
                    nc.sync.dma_start(
                        out=yt[:, offs[ci]:offs[ci] + KT * nch]
                        .rearrange("p (k n) -> p k n", k=KT),
                        in_=yq[ci][0:P])
                del res[s]
    nc.compile()
    return nc


_programs = {}


def _get_program(key, builder):
    if key not in _programs:
        _programs[key] = builder()
    return _programs[key]


def _pack_weights(params, slot_clusters):
    S = len(slot_clusters)
    wpk = np.zeros((128, S * WSLOT), np.float32)
    bpk = np.zeros((128, S * BSLOT), np.float32)
    for s, c in enumerate(slot_clusters):
        _pack_cluster(wpk, bpk, params, s, c)
    return wpk, bpk


def _route(labels, mode):
    counts = np.bincount(labels, minlength=K)
    configs = _CONFIGS if mode == "bf16" else _CONFIGS[1:]
    for S, R in configs:
        need = int(np.sum((counts + R - 1) // R))
        if need <= N_CORES * S:
            break
    nslots = N_CORES * S
    order = np.argsort(labels, kind="stable")
    slot_cluster = np.zeros(nslots, np.int64)
    slot_rows = [np.empty(0, np.int64)] * nslots
    si = pos = 0
    for c in range(K):
        cnt = int(counts[c])
        rows_c = order[pos:pos + cnt]
        pos += cnt
        for off in range(0, cnt, R):
            slot_cluster[si] = c
            slot_rows[si] = rows_c[off:off + R]
            si += 1
    return S, R, slot_cluster, slot_rows


def _flatten_xcore(xcore_t, R, chunks):
    ncols = xcore_t.shape[1]
    S = ncols // R
    flat = np.empty((P, ncols * KT), np.float32)
    pos = 0
    for s in range(S):
        col = s * R
        for nch in chunks:
            blk = xcore_t[:, col:col + nch]
            blk = blk.reshape(KT, P, nch).transpose(1, 0, 2)
            flat[:, pos:pos + KT * nch] = blk.reshape(P, KT * nch)
            pos += KT * nch
            col += nch
    return flat


def _unflatten_ycore(yflat, R, chunks):
    ncols = yflat.shape[1] // KT
    S = ncols // R
    out = np.empty((ncols, D), np.float32)
    pos = 0
    for s in range(S):
        col = s * R
        for nch in chunks:
            blk = yflat[:, pos:pos + KT * nch].reshape(P, KT, nch)
            out[col:col + nch] = blk.transpose(2, 1, 0).reshape(nch, D)
            pos += KT * nch
            col += nch
    return out


def _run_structured(x, params, strat, trace):
    import ml_dtypes
    caps, core_clusters, slot_rows = strat
    nc = _get_program(("st",) + tuple(caps),
                      lambda: _build_program_structured(caps))
    ncols = sum(caps)
    in_maps = []
    for i in range(N_CORES):
        xcore = np.zeros((ncols, D), np.float32)
        col = 0
        for s in range(len(caps)):
            rows = slot_rows[i][s]
            if len(rows):
                xcore[col:col + len(rows)] = x[rows]
            col += caps[s]
        wpk = np.zeros((128, 2 * WSLOT), np.float32)
        bpk = np.zeros((128, 2 * BSLOT), np.float32)
        _pack_cluster(wpk, bpk, params, 0, core_clusters[i][0])
        _pack_cluster(wpk, bpk, params, 1, core_clusters[i][1])
        xflat = _flatten_x_structured(np.ascontiguousarray(xcore.T), caps)
        in_maps.append({"xt": xflat.astype(ml_dtypes.bfloat16),
                        "wp": wpk.astype(ml_dtypes.bfloat16),
                        "bp": bpk})
    res = run_bass_kernel_spmd(nc, in_maps, core_ids=list(range(N_CORES)),
                               trace=trace)
    out = np.zeros_like(x)
    for i in range(N_CORES):
        yraw = np.asarray(res.results[i]["yt"]).astype(np.float32)
        ytT = _unflatten_y_structured(yraw, caps)
        col = 0
        for s in range(len(caps)):
            rows = slot_rows[i][s]
            if len(rows):
                out[rows] = ytT[col:col + len(rows)]
            col += caps[s]
    return out, res


def _run_generic(x, params, labels, mode, trace):
    S, R, slot_cluster, slot_rows = _route(labels, mode)
    chunks = _chunks(R, mode)
    nc = _get_program((S, R, mode), lambda: _build_program(S, R, mode))
    in_maps = []
    for i in range(N_CORES):
        xcore = np.zeros((S * R, D), np.float32)
        for s in range(S):
            rows = slot_rows[i * S + s]
            if len(rows):
                xcore[s * R: s * R + len(rows)] = x[rows]
        wpk, bpk = _pack_weights(params, slot_cluster[i * S:(i + 1) * S])
        xflat = _flatten_xcore(np.ascontiguousarray(xcore.T), R, chunks)
        if mode == "bf16":
            import ml_dtypes
            xflat = xflat.astype(ml_dtypes.bfloat16)
            wpk = wpk.astype(ml_dtypes.bfloat16)
        in_maps.append({"xt": xflat, "wp": wpk, "bp": bpk})
    res = run_bass_kernel_spmd(nc, in_maps, core_ids=list(range(N_CORES)),
                               trace=trace)
    out = np.zeros_like(x)
    for i in range(N_CORES):
        yraw = np.asarray(res.results[i]["yt"]).astype(np.float32)
        ytT = _unflatten_ycore(yraw, R, chunks)
        for s in range(S):
            rows = slot_rows[i * S + s]
            if len(rows):
                out[rows] = ytT[s * R: s * R + len(rows)]
    return out, res


def kernel_traced(inputs, trace=False, mode=None):
    if mode is None:
        mode = MODE
    x = np.ascontiguousarray(np.asarray(inputs["x"], dtype=np.float32))
    labels = np.asarray(inputs["kmeans_label"]).astype(np.int64).ravel()
    params = {k: np.asarray(v, dtype=np.float32)
              for k, v in inputs.items() if k not in ("x", "kmeans_label")}

    if mode == "bf16":
        strat = _route_structured(labels)
        if strat is not None:
            return _run_structured(x, params, strat, trace)
    return _run_generic(x, params, labels, mode, trace)


def kernel(**inputs):
    out, _ = kernel_traced(inputs, trace=False)
    return out

